# revision 1
# baseline (speedup 1.0000x reference)
"""GCN block (GCNConv + LayerNorm + ReLU) on 8 Trainium2 NeuronCores.

Strategy (matches the "shard nodes / partition edges by destination" hint):
  - out = LN(A_norm @ (x @ W^T) + b) = LN((A_norm @ x) @ W^T + b): aggregate
    raw features first (A_norm commutes with the linear map), so the random
    gather runs on node-major x and no transposes are needed anywhere.
  - Destination nodes are sharded contiguously across the 8 cores
    (6250 rows each); each core processes the edges that point into its
    shard.  x is replicated in every core's DRAM as two bf16 gather tables
    (even/odd node rows, so row indices fit dma_gather's int16 indices).
  - Edges are bucketed per 128-destination-node block and padded to whole
    128-edge tiles; multi-block chunks of source rows are fetched with one
    dma_gather per table (output lands tile-major: row j -> partition j%128,
    chunk j//128).  For each 128-edge tile a [128e x 128d] selection matrix
    S (S[e, d] = norm_e if dst_e == d) is built with one fused DVE
    tensor_scalar (iota == dstcol) * norm; the scatter-add is then
    G_cblk^T @ S accumulated in PSUM over the block's tiles, which directly
    yields agg^T laid out as [channel, dst] — exactly the stationary operand
    the W-matmul wants.  agg^T @ W^T gives [dst, out_ch] node-major, and
    bias + LayerNorm + ReLU are fused on DVE/ACT before a contiguous store.
"""

import math
import sys

sys.path.insert(0, "/opt/trn_rl_repo")

import numpy as np
import ml_dtypes

N_NODES = 50000
WIDTH = 256
N_CORES = 8
NODES_PER_CORE = N_NODES // N_CORES  # 6250
P = 128
N_BLOCKS = math.ceil(NODES_PER_CORE / P)  # 49 (last block has 106 rows)
LN_EPS = 1e-5
HALF = N_NODES // 2  # rows per gather table

USE_BF16 = True
GATHER_TILE_CAP = 8  # max tiles (128 idxs each) per dma_gather call (HW limit 1024)


def _preprocess(edge_index):
    """Bucket messages by (core, dst-block, src-parity table), pad each bucket
    to whole 128-edge tiles.

    Processing tile order: per block, even-table tiles then odd-table tiles.
    Gather order: even tiles of all blocks concatenated (ditto odd).
    Returns (TL, TH, dstcol[8,P,Ttot], normv[8,P,Ttot],
             idxe[8,128,8*sum(TL)] i16, idxo[8,128,8*sum(TH)] i16).
    """
    src = np.asarray(edge_index[0]).astype(np.int64)
    dst = np.asarray(edge_index[1]).astype(np.int64)
    loops = np.arange(N_NODES, dtype=np.int64)
    msrc = np.concatenate([src, loops])
    mdst = np.concatenate([dst, loops])

    deg = np.bincount(mdst, minlength=N_NODES).astype(np.float64)
    dinv = 1.0 / np.sqrt(deg)  # deg >= 1 thanks to self loops
    norm = (dinv[msrc] * dinv[mdst]).astype(np.float32)

    core = mdst // NODES_PER_CORE
    r = mdst % NODES_PER_CORE
    blk = np.minimum(r // P, N_BLOCKS - 1)
    dcol = (r - blk * P).astype(np.float32)
    tab = msrc & 1
    gbin = (core * N_BLOCKS + blk) * 2 + tab

    order = np.argsort(gbin, kind="stable")
    msrc, norm, dcol, gbin = msrc[order], norm[order], dcol[order], gbin[order]

    cnt = np.bincount(gbin, minlength=N_CORES * N_BLOCKS * 2).reshape(
        N_CORES, N_BLOCKS, 2
    )
    TL = [int(math.ceil(int(cnt[:, b, 0].max()) / P)) for b in range(N_BLOCKS)]
    TH = [int(math.ceil(int(cnt[:, b, 1].max()) / P)) for b in range(N_BLOCKS)]
    sTL, sTH = sum(TL), sum(TH)
    Ttot = sTL + sTH
    # tile offsets
    EOFF = np.concatenate([[0], np.cumsum(TL)])  # even gather order
    OOFF = np.concatenate([[0], np.cumsum(TH)])  # odd gather order
    TOFF = np.concatenate([[0], np.cumsum(np.asarray(TL) + np.asarray(TH))])

    dstcol = np.zeros((N_CORES, P, Ttot), np.float32)
    normv = np.zeros((N_CORES, P, Ttot), np.float32)
    idxe_flat = np.zeros((N_CORES, sTL * P), np.int16)
    idxo_flat = np.zeros((N_CORES, sTH * P), np.int16)

    starts = np.concatenate([[0], np.cumsum(cnt.ravel())])[:-1]
    j = np.arange(len(gbin)) - starts[gbin]  # index within bucket
    c = gbin // (N_BLOCKS * 2)
    b = (gbin // 2) % N_BLOCKS
    t = gbin & 1
    tile_in_bucket = j // P
    p = j % P
    # metadata in processing order
    tg = np.where(
        t == 0,
        TOFF[b] + tile_in_bucket,
        TOFF[b] + np.asarray(TL)[b] + tile_in_bucket,
    )
    dstcol[c, p, tg] = dcol
    normv[c, p, tg] = norm
    # gather index arrays (per-table tile order)
    idx16 = (msrc >> 1).astype(np.int16)
    Je = (EOFF[b] + tile_in_bucket) * P + p
    Jo = (OOFF[b] + tile_in_bucket) * P + p
    ev = t == 0
    idxe_flat[c[ev], Je[ev]] = idx16[ev]
    idxo_flat[c[~ev], Jo[~ev]] = idx16[~ev]

    # wrap: flat j -> (partition j%16, column j//16), replicated on 8 stripes
    def wrap(flat, ntiles):
        if ntiles == 0:
            return np.zeros((N_CORES, P, 0), np.int16)
        a = flat.reshape(N_CORES, ntiles * 8, 16).transpose(0, 2, 1)  # [8,16,cols]
        return np.ascontiguousarray(np.tile(a, (1, 8, 1)))  # [8,128,cols]

    return TL, TH, dstcol, normv, wrap(idxe_flat, sTL), wrap(idxo_flat, sTH)


def _chunks(TL, TH):
    """Group consecutive blocks into gather chunks where EACH table's tile
    count stays within one dma_gather call's limit."""
    out = []
    cur = []
    ne = no = 0
    for b in range(N_BLOCKS):
        if cur and (ne + TL[b] > GATHER_TILE_CAP or no + TH[b] > GATHER_TILE_CAP):
            out.append((cur, ne, no))
            cur, ne, no = [], 0, 0
        cur.append(b)
        ne += TL[b]
        no += TH[b]
    if cur:
        out.append((cur, ne, no))
    return out


def _build_program(TL, TH, generic_affine, bias_mean):
    import concourse.bass as bass
    import concourse.tile as tile
    from concourse import bacc as bacc_mod
    from concourse import mybir
    from contextlib import ExitStack

    f32 = mybir.dt.float32
    bf16 = mybir.dt.bfloat16
    cdt = bf16 if USE_BF16 else f32
    i16 = mybir.dt.int16
    Alu = mybir.AluOpType
    Act = mybir.ActivationFunctionType
    sTL, sTH = sum(TL), sum(TH)
    Ttot = sTL + sTH
    EOFF = np.concatenate([[0], np.cumsum(TL)])
    OOFF = np.concatenate([[0], np.cumsum(TH)])
    chunks = _chunks(TL, TH)
    max_ne = max(ch[1] for ch in chunks)
    max_no = max(ch[2] for ch in chunks)

    # fcon (f32) column layout: [dst | norm | bias | gamma? | beta?]
    FW = 2 * Ttot + WIDTH + (2 * WIDTH if generic_affine else 0)
    # bcon (cdt) column layout:  [wt_ext 2*(WIDTH+1) | iota (P)]
    BW = 2 * (WIDTH + 1) + P

    nc = bacc_mod.Bacc(None, target_bir_lowering=False, debug=False, num_swdge_queues=4)
    xe_d = nc.declare_dram_parameter("xe", [HALF, WIDTH], cdt, isOutput=False)
    xo_d = nc.declare_dram_parameter("xo", [HALF, WIDTH], cdt, isOutput=False)
    idxe_d = nc.declare_dram_parameter("idxe", [P, 8 * sTL], i16, isOutput=False)
    idxo_d = nc.declare_dram_parameter("idxo", [P, 8 * sTH], i16, isOutput=False)
    fcon_d = nc.declare_dram_parameter("fcon", [P, FW], f32, isOutput=False)
    bcon_d = nc.declare_dram_parameter("bcon", [P, BW], cdt, isOutput=False)
    out_d = nc.declare_dram_parameter("out", [NODES_PER_CORE, WIDTH], f32, isOutput=True)

    with tile.TileContext(nc) as tc:
        with ExitStack() as ctx:
            const = ctx.enter_context(tc.tile_pool(name="const", bufs=1))
            gpool = ctx.enter_context(tc.tile_pool(name="g", bufs=2))
            spool = ctx.enter_context(tc.tile_pool(name="s", bufs=6))
            apool = ctx.enter_context(tc.tile_pool(name="aggT", bufs=2))
            ypool = ctx.enter_context(tc.tile_pool(name="y", bufs=2))
            stat = ctx.enter_context(tc.tile_pool(name="stat", bufs=4))
            ppool = ctx.enter_context(tc.tile_pool(name="psA", bufs=2, space="PSUM"))
            opsum = ctx.enter_context(tc.tile_pool(name="psO", bufs=2, space="PSUM"))

            idxe_sb = const.tile([P, 8 * sTL], i16)
            nc.sync.dma_start(idxe_sb[:], idxe_d[:, :])
            idxo_sb = const.tile([P, 8 * sTH], i16)
            nc.sync.dma_start(idxo_sb[:], idxo_d[:, :])
            fcon_sb = const.tile([P, FW], f32)
            nc.sync.dma_start(fcon_sb[:], fcon_d[:, :])
            bcon_sb = const.tile([P, BW], cdt)
            nc.sync.dma_start(bcon_sb[:], bcon_d[:, :])
            eps_sb = const.tile([P, 1], f32)
            nc.vector.memset(eps_sb[:], LN_EPS)

            bias_sb = fcon_sb[:, 2 * Ttot : 2 * Ttot + WIDTH]
            if generic_affine:
                gamma_sb = fcon_sb[:, 2 * Ttot + WIDTH : 2 * Ttot + 2 * WIDTH]
                beta_sb = fcon_sb[:, 2 * Ttot + 2 * WIDTH : 2 * Ttot + 3 * WIDTH]
            wt_sb = bcon_sb[:, : 2 * (WIDTH + 1)]
            iota_sb = bcon_sb[:, 2 * (WIDTH + 1) : 2 * (WIDTH + 1) + P]
            bmean_sb = const.tile([P, 1], f32)
            nc.vector.memset(bmean_sb[:], bias_mean)

            qn = 0
            for blocks, ne, no in chunks:
                e0 = int(EOFF[blocks[0]])
                o0 = int(OOFF[blocks[0]])
                ge = go = None
                if ne:
                    ge = gpool.tile([P, ne, WIDTH], cdt, tag="ge")
                    nc.gpsimd.dma_gather(
                        ge[:],
                        xe_d[:, :],
                        idxe_sb[:, 8 * e0 : 8 * (e0 + ne)],
                        ne * P,
                        ne * P,
                        WIDTH,
                        queue_num=qn % 4,
                    )
                    qn += 1
                if no:
                    go = gpool.tile([P, no, WIDTH], cdt, tag="go")
                    nc.gpsimd.dma_gather(
                        go[:],
                        xo_d[:, :],
                        idxo_sb[:, 8 * o0 : 8 * (o0 + no)],
                        no * P,
                        no * P,
                        WIDTH,
                        queue_num=qn % 4,
                    )
                    qn += 1
                for b in blocks:
                    tg0 = int(
                        np.concatenate([[0], np.cumsum(np.asarray(TL) + np.asarray(TH))])[
                            b
                        ]
                    )
                    seq = [(ge, int(EOFF[b]) - e0 + t) for t in range(TL[b])] + [
                        (go, int(OOFF[b]) - o0 + t) for t in range(TH[b])
                    ]
                    nt = len(seq)
                    ps0 = ppool.tile([P, P], f32, tag="ps0")
                    ps1 = ppool.tile([P, P], f32, tag="ps1")
                    for k, (gt, col) in enumerate(seq):
                        tg = tg0 + k
                        s = spool.tile([P, P], cdt, tag="s")
                        nc.vector.tensor_scalar(
                            out=s[:],
                            in0=iota_sb,
                            scalar1=fcon_sb[:, tg : tg + 1],
                            scalar2=fcon_sb[:, Ttot + tg : Ttot + tg + 1],
                            op0=Alu.is_equal,
                            op1=Alu.mult,
                        )
                        nc.tensor.matmul(
                            out=ps0[:],
                            lhsT=gt[:, col, 0:P],
                            rhs=s[:],
                            start=(k == 0),
                            stop=(k == nt - 1),
                        )
                        nc.tensor.matmul(
                            out=ps1[:],
                            lhsT=gt[:, col, P:WIDTH],
                            rhs=s[:],
                            start=(k == 0),
                            stop=(k == nt - 1),
                        )
                    # aggT blocks [128 ch, 128 dst] -> SBUF (cast) for W-matmul
                    a0 = apool.tile([P, P], cdt, tag="a0")
                    nc.scalar.copy(a0[:], ps0[:])
                    a1 = apool.tile([P, P], cdt, tag="a1")
                    nc.scalar.copy(a1[:], ps1[:])
                    po = opsum.tile([P, WIDTH + 1], f32, tag="po")
                    nc.tensor.matmul(
                        out=po[:],
                        lhsT=a0[:],
                        rhs=wt_sb[:, : WIDTH + 1],
                        start=True,
                        stop=False,
                    )
                    nc.tensor.matmul(
                        out=po[:],
                        lhsT=a1[:],
                        rhs=wt_sb[:, WIDTH + 1 :],
                        start=False,
                        stop=True,
                    )
                    # ---- epilogue: y = po + bias; LayerNorm; ReLU ----
                    y = ypool.tile([P, WIDTH], f32, tag="y")
                    # NOTE: tensor_tensor_reduce hard-crashes TRN2 here; plain
                    # add, with the row-sum coming free from the W-matmul's
                    # extra weight column (po[:, WIDTH]).
                    nc.vector.tensor_tensor(
                        out=y[:], in0=po[:, :WIDTH], in1=bias_sb, op=Alu.add
                    )
                    sq = ypool.tile([P, WIDTH], f32, tag="sq")
                    ssq = stat.tile([P, 1], f32, tag="ssq")
                    nc.scalar.activation(
                        out=sq[:], in_=y[:], func=Act.Square, accum_out=ssq[:]
                    )
                    mu = stat.tile([P, 1], f32, tag="mu")
                    nc.scalar.activation(
                        out=mu[:],
                        in_=po[:, WIDTH : WIDTH + 1],
                        func=Act.Identity,
                        scale=1.0 / WIDTH,
                        bias=bmean_sb[:, :1],
                    )
                    m2 = stat.tile([P, 1], f32, tag="m2")
                    nc.scalar.square(m2[:], mu[:])
                    var = stat.tile([P, 1], f32, tag="var")
                    nc.vector.tensor_scalar(
                        out=var[:],
                        in0=ssq[:],
                        scalar1=1.0 / WIDTH,
                        scalar2=m2[:, :1],
                        op0=Alu.mult,
                        op1=Alu.subtract,
                    )
                    sd = stat.tile([P, 1], f32, tag="sd")
                    nc.scalar.activation(
                        out=sd[:], in_=var[:], func=Act.Sqrt, bias=eps_sb[:, :1]
                    )
                    rstd = stat.tile([P, 1], f32, tag="rstd")
                    nc.vector.reciprocal(rstd[:], sd[:])
                    t1 = ypool.tile([P, WIDTH], f32, tag="t1")
                    nc.vector.tensor_scalar(
                        out=t1[:],
                        in0=y[:],
                        scalar1=mu[:, :1],
                        scalar2=rstd[:, :1],
                        op0=Alu.subtract,
                        op1=Alu.mult,
                    )
                    if generic_affine:
                        t2 = ypool.tile([P, WIDTH], f32, tag="t2")
                        nc.vector.tensor_tensor(
                            out=t2[:], in0=t1[:], in1=gamma_sb, op=Alu.mult
                        )
                        t3 = ypool.tile([P, WIDTH], f32, tag="t3")
                        nc.vector.tensor_tensor(
                            out=t3[:], in0=t2[:], in1=beta_sb, op=Alu.add
                        )
                        t1 = t3
                    yo = ypool.tile([P, WIDTH], f32, tag="yo")
                    nc.scalar.activation(out=yo[:], in_=t1[:], func=Act.Relu)
                    rows = min(P, NODES_PER_CORE - b * P)
                    nc.sync.dma_start(out_d[b * P : b * P + rows, :], yo[:rows, :])
    return nc


def _pack_inputs(TL, TH, dstcol, normv, idxe, idxo, x, W, bias, gamma, beta, generic_affine):
    cnp = ml_dtypes.bfloat16 if USE_BF16 else np.float32
    Ttot = sum(TL) + sum(TH)

    xc = x.astype(cnp)
    xe = np.ascontiguousarray(xc[0::2])
    xo = np.ascontiguousarray(xc[1::2])
    WT32 = W.T.astype(np.float32)  # [in, out]
    rs = WT32.sum(axis=1, keepdims=True)  # [256, 1] row sums
    WTe = np.concatenate([WT32, rs], axis=1).astype(cnp)  # [256, 257]
    wt = np.concatenate([WTe[:P], WTe[P:]], axis=1)  # [128, 514]
    iota = np.tile(np.arange(P), (P, 1)).astype(cnp)
    bcon = np.ascontiguousarray(np.concatenate([wt, iota], axis=1))

    biasb = np.tile(bias.astype(np.float32)[None, :], (P, 1))
    fparts = [None, None, biasb]
    if generic_affine:
        fparts.append(np.tile(gamma.astype(np.float32)[None, :], (P, 1)))
        fparts.append(np.tile(beta.astype(np.float32)[None, :], (P, 1)))

    in_maps = []
    for c in range(N_CORES):
        fparts[0] = dstcol[c]
        fparts[1] = normv[c]
        fcon = np.ascontiguousarray(np.concatenate(fparts, axis=1, dtype=np.float32))
        in_maps.append(
            {
                "xe": xe,
                "xo": xo,
                "idxe": np.ascontiguousarray(idxe[c]),
                "idxo": np.ascontiguousarray(idxo[c]),
                "fcon": fcon,
                "bcon": bcon,
            }
        )
    return in_maps


_PROGRAM_CACHE = {}


def kernel(x, edge_index, W, b, gamma, beta, _run_kwargs=None):
    from concourse.bass_utils import run_bass_kernel_spmd

    x = np.asarray(x)
    W = np.asarray(W)
    bias = np.asarray(b)
    gamma = np.asarray(gamma)
    beta = np.asarray(beta)

    TL, TH, dstcol, normv, idxe, idxo = _preprocess(edge_index)
    generic_affine = not (np.all(gamma == 1.0) and np.all(beta == 0.0))

    bias_mean = float(bias.astype(np.float64).mean())
    key = (tuple(TL), tuple(TH), generic_affine, bias_mean)
    if key not in _PROGRAM_CACHE:
        nc = _build_program(TL, TH, generic_affine, bias_mean)
        nc.finalize()
        _PROGRAM_CACHE[key] = nc
    nc = _PROGRAM_CACHE[key]

    in_maps = _pack_inputs(
        TL, TH, dstcol, normv, idxe, idxo, x, W, bias, gamma, beta, generic_affine
    )

    kwargs = dict(_run_kwargs or {})
    kwargs.pop("_result", None)
    rr = run_bass_kernel_spmd(nc, in_maps, list(range(N_CORES)), **kwargs)
    out = np.concatenate([rr.results[c]["out"] for c in range(N_CORES)], axis=0)
    if _run_kwargs is not None:
        _run_kwargs["_result"] = rr
    return np.ascontiguousarray(out.astype(np.float32))



# revision 3
# speedup vs baseline: 1.1905x; 1.1905x over previous
"""GCN block (GCNConv + LayerNorm + ReLU) on 8 Trainium2 NeuronCores.

Strategy v2 (gather-descriptor-rate aware):
  - out = LN((A_norm @ x) @ W^T + b): aggregate raw features first, so the
    random gather runs on node-major x.
  - The SWDGE gather is descriptor-generation-bound (~2.9 ns/row across the
    4 ucode queues), so the kernel minimizes gathered rows:
      * self-loops are NOT gathered: their diag(dinv^2) x W^T term is a dense
        matmul against an SBUF-resident transposed shard copy;
      * destination nodes are bin-packed (host side) into 49 blocks per core
        with edge counts just under a multiple of 128, so tile padding is ~1%
        instead of ~25%;
      * per-core gather tables hold only the ~27k unique source rows a core
        needs, so int16 indices cover them without an even/odd table split.
  - norm factorizes: dinv[src] is folded into the gather table rows on the
    host; dinv[dst] is applied per destination block in the epilogue.  The
    per-tile scatter matrices S (0/1) are therefore input data: they are
    DMA-ed in (bf16) rather than built on the vector engine, which removes
    the old per-tile DVE bottleneck entirely.
  - Per block: PSUM accumulates aggT = sum_tiles G^T S ([ch, dst]), then
    po = aggT^T @ WTe + xTself_blk^T @ WTe (WTe carries an extra row-sum
    column so the LN mean is free), and the epilogue does
    relu(LN(po * dinv_d + bias)) fused on DVE/ACT before a contiguous store.
"""

import math
import sys

sys.path.insert(0, "/opt/trn_rl_repo")

import numpy as np
import ml_dtypes

N_NODES = 50000
N_EDGES = 312500
WIDTH = 256
N_CORES = 8
P = 128
N_BINS = 49
NODES_PER_CORE = N_NODES // N_CORES  # 6250
DST_SLOTS = N_BINS * P  # 6272 padded dst slots per core
LN_EPS = 1e-5
TABLE_ROWS = 32768


def _shard_nodes(deg_in):
    """Assign each node to a core (exactly NODES_PER_CORE each), balancing
    total edge counts.  Greedy over nodes sorted by in-degree."""
    import heapq

    order = np.argsort(-deg_in, kind="stable")
    node_core = np.empty(N_NODES, np.int32)
    cnt = np.zeros(N_CORES, np.int64)
    heap = [(0, c) for c in range(N_CORES)]
    heapq.heapify(heap)
    for n in order:
        while True:
            e, c = heapq.heappop(heap)
            if cnt[c] < NODES_PER_CORE:
                break
        node_core[n] = c
        cnt[c] += 1
        if cnt[c] < NODES_PER_CORE:
            heapq.heappush(heap, (e + int(deg_in[n]), c))
    return node_core


def _pack_bins(nodes, degs, targets):
    """Greedy bin-pack `nodes` (with edge counts `degs`) into len(targets)
    bins of <=P nodes and ~targets[b] edges.  Returns bin id per node."""
    nb = len(targets)
    rem_e = np.asarray(targets, np.int64).copy()
    rem_s = np.full(nb, P, np.int64)
    order = np.argsort(-degs, kind="stable")
    bin_of = np.empty(len(nodes), np.int32)
    for i in order:
        d = degs[i]
        open_b = np.flatnonzero(rem_s > 0)
        fits = open_b[rem_e[open_b] >= d]
        b = (fits if len(fits) else open_b)[
            np.argmax(rem_e[fits if len(fits) else open_b])
        ]
        bin_of[i] = b
        rem_e[b] -= d
        rem_s[b] -= 1
    return bin_of


def _preprocess(edge_index, x, dinv):
    """Returns (profile, per-core input dict pieces, out node mapping)."""
    src = np.asarray(edge_index[0]).astype(np.int64)
    dst = np.asarray(edge_index[1]).astype(np.int64)
    deg_in = np.bincount(dst, minlength=N_NODES)

    node_core = _shard_nodes(deg_in)
    core_edges = np.bincount(node_core[dst], minlength=N_CORES)
    m = int(core_edges.max())
    tt = math.ceil(m / P) + 2
    n7 = tt - 6 * N_BINS
    assert 0 < n7 <= N_BINS, (tt, n7)
    targets = np.array([7 * P] * n7 + [6 * P] * (N_BINS - n7), np.int64)

    cores = []
    node_slot = np.empty(N_NODES, np.int64)  # slot (0..6271) within its core
    for c in range(N_CORES):
        nodes = np.flatnonzero(node_core == c)
        degs = deg_in[nodes]
        bin_of = _pack_bins(nodes, degs, targets)
        cnt = np.bincount(bin_of, weights=degs, minlength=N_BINS).astype(np.int64)
        # order bins by edge count desc so the per-position tile profile is
        # aligned across cores
        border = np.argsort(-cnt, kind="stable")
        rank = np.empty(N_BINS, np.int64)
        rank[border] = np.arange(N_BINS)
        bin_of = rank[bin_of]
        cnt = cnt[border]
        # column within bin
        order2 = np.argsort(bin_of, kind="stable")
        col = np.empty(len(nodes), np.int64)
        col[order2] = np.arange(len(nodes)) - np.concatenate(
            [[0], np.cumsum(np.bincount(bin_of, minlength=N_BINS))]
        )[bin_of[order2]]
        assert col.max() < P
        node_slot[nodes] = bin_of * P + col
        cores.append((nodes, bin_of, col, cnt))

    prof = np.zeros(N_BINS, np.int64)
    for _, _, _, cnt in cores:
        prof = np.maximum(prof, np.ceil(cnt / P).astype(np.int64))
    profile = tuple(int(t) for t in prof)
    ttot = int(sum(profile))
    toff = np.concatenate([[0], np.cumsum(prof)])

    xd = (np.asarray(x, np.float64) * dinv[:, None]).astype(np.float32)

    in_maps = []
    for c in range(N_CORES):
        nodes, bin_of, col, cnt = cores[c]
        e_mask = node_core[dst] == c
        e_src = src[e_mask]
        e_dst = dst[e_mask]
        e_bin = node_slot[e_dst] // P
        e_col = node_slot[e_dst] % P
        eorder = np.argsort(e_bin, kind="stable")
        e_src, e_bin, e_col = e_src[eorder], e_bin[eorder], e_col[eorder]
        # position within bin -> (tile, pos)
        within = np.arange(len(e_src)) - np.concatenate(
            [[0], np.cumsum(np.bincount(e_bin, minlength=N_BINS))]
        )[e_bin]
        tile = toff[e_bin] + within // P
        pos = within % P

        uniq, inv = np.unique(e_src, return_inverse=True)
        assert len(uniq) < TABLE_ROWS, len(uniq)
        table = np.zeros((TABLE_ROWS, WIDTH), ml_dtypes.bfloat16)
        table[: len(uniq)] = xd[uniq]

        idx_flat = np.zeros(ttot * P, np.int16)
        idx_flat[tile * P + pos] = inv.astype(np.int16)
        a = idx_flat.reshape(ttot * 8, 16).transpose(1, 0)
        idx = np.ascontiguousarray(np.tile(a, (8, 1)))

        s_u16 = np.zeros((P, ttot * P), np.uint16)
        s_u16[pos, tile * P + e_col] = 0x3F80  # bf16 1.0
        s_all = s_u16.view(ml_dtypes.bfloat16)

        # per-slot node map, dinv tables, self-feature transpose
        slot_node = np.full(DST_SLOTS, -1, np.int64)
        slot_node[bin_of * P + col] = nodes
        valid = slot_node >= 0
        dinvtab = np.ones((P, N_BINS), np.float32)
        dv = np.ones(DST_SLOTS, np.float32)
        dv[valid] = dinv[slot_node[valid]].astype(np.float32)
        dinvtab = dv.reshape(N_BINS, P).transpose(1, 0).copy()
        dinvW = dinvtab / WIDTH
        xs = np.zeros((DST_SLOTS, WIDTH), np.float32)
        xs[valid] = xd[slot_node[valid]]
        xtself = np.concatenate([xs[:, :P].T, xs[:, P:].T], axis=1).astype(
            ml_dtypes.bfloat16
        )
        in_maps.append(
            {
                "tab": table,
                "idx": idx,
                "sall": np.ascontiguousarray(s_all),
                "xtself": np.ascontiguousarray(xtself),
                "dcon": np.ascontiguousarray(
                    np.concatenate([dinvtab, dinvW], axis=1)
                ),
                "_slot_node": slot_node,
            }
        )
    return profile, in_maps


def _build_program(profile, bias_mean, generic_affine):
    import concourse.bass as bass
    import concourse.tile as tile
    from concourse import bacc as bacc_mod
    from concourse import mybir
    from contextlib import ExitStack

    f32 = mybir.dt.float32
    bf16 = mybir.dt.bfloat16
    i16 = mybir.dt.int16
    Alu = mybir.AluOpType
    Act = mybir.ActivationFunctionType
    ttot = int(sum(profile))
    toff = np.concatenate([[0], np.cumsum(profile)]).astype(np.int64)

    # fcon (f32) columns: [bias (W) | gamma? | beta?]
    FW = WIDTH + (2 * WIDTH if generic_affine else 0)

    nc = bacc_mod.Bacc(None, target_bir_lowering=False, debug=False, num_swdge_queues=4)
    tab_d = nc.declare_dram_parameter("tab", [TABLE_ROWS, WIDTH], bf16, isOutput=False)
    idx_d = nc.declare_dram_parameter("idx", [P, 8 * ttot], i16, isOutput=False)
    sall_d = nc.declare_dram_parameter("sall", [P, ttot * P], bf16, isOutput=False)
    xts_d = nc.declare_dram_parameter("xtself", [P, 2 * DST_SLOTS], bf16, isOutput=False)
    dcon_d = nc.declare_dram_parameter("dcon", [P, 2 * N_BINS], f32, isOutput=False)
    wt_d = nc.declare_dram_parameter("wt", [P, 2 * (WIDTH + 1)], bf16, isOutput=False)
    fcon_d = nc.declare_dram_parameter("fcon", [P, FW], f32, isOutput=False)
    out_d = nc.declare_dram_parameter("out", [DST_SLOTS, WIDTH], f32, isOutput=True)

    with tile.TileContext(nc) as tc:
        with ExitStack() as ctx:
            const = ctx.enter_context(tc.tile_pool(name="const", bufs=1))
            gpool = ctx.enter_context(tc.tile_pool(name="g", bufs=8))
            spool = ctx.enter_context(tc.tile_pool(name="s", bufs=8))
            apool = ctx.enter_context(tc.tile_pool(name="aggT", bufs=2))
            ypool = ctx.enter_context(tc.tile_pool(name="y", bufs=3))
            stat = ctx.enter_context(tc.tile_pool(name="stat", bufs=4))
            ppool = ctx.enter_context(tc.tile_pool(name="psA", bufs=2, space="PSUM"))
            opsum = ctx.enter_context(tc.tile_pool(name="psO", bufs=2, space="PSUM"))

            idx_sb = const.tile([P, 8 * ttot], i16)
            nc.sync.dma_start(idx_sb[:], idx_d[:, :])
            wt_sb = const.tile([P, 2 * (WIDTH + 1)], bf16)
            nc.sync.dma_start(wt_sb[:], wt_d[:, :])
            fcon_sb = const.tile([P, FW], f32)
            nc.sync.dma_start(fcon_sb[:], fcon_d[:, :])
            dcon_sb = const.tile([P, 2 * N_BINS], f32)
            nc.sync.dma_start(dcon_sb[:], dcon_d[:, :])
            xts_sb = const.tile([P, 2 * DST_SLOTS], bf16)
            nc.sync.dma_start(xts_sb[:], xts_d[:, :])
            eps_sb = const.tile([P, 1], f32)
            nc.vector.memset(eps_sb[:], LN_EPS)

            bias_sb = fcon_sb[:, :WIDTH]
            if generic_affine:
                gamma_sb = fcon_sb[:, WIDTH : 2 * WIDTH]
                beta_sb = fcon_sb[:, 2 * WIDTH : 3 * WIDTH]

            for b in range(N_BINS):
                nt = int(profile[b])
                t0 = int(toff[b])
                g = gpool.tile([P, nt, WIDTH], bf16, tag="g")
                nc.gpsimd.dma_gather(
                    g[:],
                    tab_d[:, :],
                    idx_sb[:, 8 * t0 : 8 * (t0 + nt)],
                    nt * P,
                    nt * P,
                    WIDTH,
                    queue_num=b % 4,
                )
                s = spool.tile([P, nt * P], bf16, tag="s")
                nc.sync.dma_start(s[:], sall_d[:, t0 * P : (t0 + nt) * P])

                ps0 = ppool.tile([P, P], f32, tag="ps0")
                ps1 = ppool.tile([P, P], f32, tag="ps1")
                for k in range(nt):
                    nc.tensor.matmul(
                        out=ps0[:],
                        lhsT=g[:, k, 0:P],
                        rhs=s[:, k * P : (k + 1) * P],
                        start=(k == 0),
                        stop=(k == nt - 1),
                    )
                    nc.tensor.matmul(
                        out=ps1[:],
                        lhsT=g[:, k, P:WIDTH],
                        rhs=s[:, k * P : (k + 1) * P],
                        start=(k == 0),
                        stop=(k == nt - 1),
                    )
                a0 = apool.tile([P, P], bf16, tag="a0")
                nc.scalar.copy(a0[:], ps0[:])
                a1 = apool.tile([P, P], bf16, tag="a1")
                nc.scalar.copy(a1[:], ps1[:])
                po = opsum.tile([P, WIDTH + 1], f32, tag="po")
                nc.tensor.matmul(
                    out=po[:], lhsT=a0[:], rhs=wt_sb[:, : WIDTH + 1],
                    start=True, stop=False,
                )
                nc.tensor.matmul(
                    out=po[:], lhsT=a1[:], rhs=wt_sb[:, WIDTH + 1 :],
                    start=False, stop=False,
                )
                nc.tensor.matmul(
                    out=po[:],
                    lhsT=xts_sb[:, b * P : (b + 1) * P],
                    rhs=wt_sb[:, : WIDTH + 1],
                    start=False, stop=False,
                )
                nc.tensor.matmul(
                    out=po[:],
                    lhsT=xts_sb[:, DST_SLOTS + b * P : DST_SLOTS + (b + 1) * P],
                    rhs=wt_sb[:, WIDTH + 1 :],
                    start=False, stop=True,
                )
                # ---- epilogue: y = po*dinv_d + bias; LayerNorm; ReLU ----
                ys = ypool.tile([P, WIDTH], f32, tag="ys")
                nc.vector.tensor_scalar(
                    out=ys[:], in0=po[:, :WIDTH],
                    scalar1=dcon_sb[:, b : b + 1], scalar2=None, op0=Alu.mult,
                )
                y = ypool.tile([P, WIDTH], f32, tag="y")
                nc.vector.tensor_tensor(out=y[:], in0=ys[:], in1=bias_sb, op=Alu.add)
                sq = ypool.tile([P, WIDTH], f32, tag="sq")
                ssq = stat.tile([P, 1], f32, tag="ssq")
                nc.scalar.activation(
                    out=sq[:], in_=y[:], func=Act.Square, accum_out=ssq[:]
                )
                mu = stat.tile([P, 1], f32, tag="mu")
                nc.vector.tensor_scalar(
                    out=mu[:],
                    in0=po[:, WIDTH : WIDTH + 1],
                    scalar1=dcon_sb[:, N_BINS + b : N_BINS + b + 1],
                    scalar2=float(bias_mean),
                    op0=Alu.mult,
                    op1=Alu.add,
                )
                m2 = stat.tile([P, 1], f32, tag="m2")
                nc.scalar.square(m2[:], mu[:])
                var = stat.tile([P, 1], f32, tag="var")
                nc.vector.tensor_scalar(
                    out=var[:],
                    in0=ssq[:],
                    scalar1=1.0 / WIDTH,
                    scalar2=m2[:, :1],
                    op0=Alu.mult,
                    op1=Alu.subtract,
                )
                sd = stat.tile([P, 1], f32, tag="sd")
                nc.scalar.activation(
                    out=sd[:], in_=var[:], func=Act.Sqrt, bias=eps_sb[:, :1]
                )
                rstd = stat.tile([P, 1], f32, tag="rstd")
                nc.vector.reciprocal(rstd[:], sd[:])
                t1 = ypool.tile([P, WIDTH], f32, tag="t1")
                nc.vector.tensor_scalar(
                    out=t1[:],
                    in0=y[:],
                    scalar1=mu[:, :1],
                    scalar2=rstd[:, :1],
                    op0=Alu.subtract,
                    op1=Alu.mult,
                )
                if generic_affine:
                    t2 = ypool.tile([P, WIDTH], f32, tag="t2")
                    nc.vector.tensor_tensor(out=t2[:], in0=t1[:], in1=gamma_sb, op=Alu.mult)
                    t3 = ypool.tile([P, WIDTH], f32, tag="t3")
                    nc.vector.tensor_tensor(out=t3[:], in0=t2[:], in1=beta_sb, op=Alu.add)
                    t1 = t3
                yo = ypool.tile([P, WIDTH], f32, tag="yo")
                nc.scalar.activation(out=yo[:], in_=t1[:], func=Act.Relu)
                nc.sync.dma_start(out_d[b * P : (b + 1) * P, :], yo[:])
    return nc


_PROGRAM_CACHE = {}
_PREP_CACHE = {}


def kernel(x, edge_index, W, b, gamma, beta, _run_kwargs=None):
    from concourse.bass_utils import run_bass_kernel_spmd

    x = np.asarray(x)
    W = np.asarray(W)
    bias = np.asarray(b, dtype=np.float64)
    gamma = np.asarray(gamma)
    beta = np.asarray(beta)

    ekey = hash(np.asarray(edge_index).tobytes()) ^ hash(x.tobytes())
    if ekey not in _PREP_CACHE:
        dst = np.asarray(edge_index[1]).astype(np.int64)
        deg = np.bincount(dst, minlength=N_NODES).astype(np.float64) + 1.0
        dinv = 1.0 / np.sqrt(deg)
        _PREP_CACHE.clear()
        _PREP_CACHE[ekey] = _preprocess(edge_index, x, dinv)
    profile, in_maps = _PREP_CACHE[ekey]

    generic_affine = not (np.all(gamma == 1.0) and np.all(beta == 0.0))
    bias_mean = float(bias.mean())
    key = (profile, generic_affine, bias_mean)
    if key not in _PROGRAM_CACHE:
        nc = _build_program(profile, bias_mean, generic_affine)
        nc.finalize()
        _PROGRAM_CACHE[key] = nc
    nc = _PROGRAM_CACHE[key]

    WT32 = W.T.astype(np.float32)
    rs = WT32.sum(axis=1, keepdims=True)
    WTe = np.concatenate([WT32, rs], axis=1).astype(ml_dtypes.bfloat16)
    wt = np.ascontiguousarray(np.concatenate([WTe[:P], WTe[P:]], axis=1))
    fparts = [np.tile(bias.astype(np.float32)[None, :], (P, 1))]
    if generic_affine:
        fparts.append(np.tile(gamma.astype(np.float32)[None, :], (P, 1)))
        fparts.append(np.tile(beta.astype(np.float32)[None, :], (P, 1)))
    fcon = np.ascontiguousarray(np.concatenate(fparts, axis=1, dtype=np.float32))

    run_maps = []
    for c in range(N_CORES):
        m = {k: v for k, v in in_maps[c].items() if not k.startswith("_")}
        m["wt"] = wt
        m["fcon"] = fcon
        run_maps.append(m)

    kwargs = dict(_run_kwargs or {})
    kwargs.pop("_result", None)
    rr = run_bass_kernel_spmd(nc, run_maps, list(range(N_CORES)), **kwargs)
    out = np.zeros((N_NODES, WIDTH), np.float32)
    for c in range(N_CORES):
        slot_node = in_maps[c]["_slot_node"]
        valid = slot_node >= 0
        out[slot_node[valid]] = rr.results[c]["out"][valid]
    if _run_kwargs is not None:
        _run_kwargs["_result"] = rr
    return out


# revision 11
# speedup vs baseline: 1.8120x; 1.5221x over previous
"""GCN block (GCNConv + LayerNorm + ReLU) on 8 Trainium2 NeuronCores.

Strategy v2 (gather-descriptor-rate aware):
  - out = LN((A_norm @ x) @ W^T + b): aggregate raw features first, so the
    random gather runs on node-major x.
  - The SWDGE gather is descriptor-generation-bound (~2.9 ns/row across the
    4 ucode queues), so the kernel minimizes gathered rows:
      * self-loops are NOT gathered: their diag(dinv^2) x W^T term is a dense
        matmul against an SBUF-resident transposed shard copy;
      * destination nodes are bin-packed (host side) into 49 blocks per core
        with edge counts just under a multiple of 128, so tile padding is ~1%
        instead of ~25%;
      * per-core gather tables hold only the ~27k unique source rows a core
        needs, so int16 indices cover them without an even/odd table split.
  - norm factorizes: dinv[src] is folded into the gather table rows on the
    host; dinv[dst] is applied per destination block in the epilogue.  The
    per-tile scatter matrices S (0/1) are therefore input data: they are
    DMA-ed in (bf16) rather than built on the vector engine, which removes
    the old per-tile DVE bottleneck entirely.
  - Per block: PSUM accumulates aggT = sum_tiles G^T S ([ch, dst]), then
    po = aggT^T @ WTe + xTself_blk^T @ WTe (WTe carries an extra row-sum
    column so the LN mean is free), and the epilogue does
    relu(LN(po * dinv_d + bias)) fused on DVE/ACT before a contiguous store.
"""

import math
import sys

sys.path.insert(0, "/opt/trn_rl_repo")

import numpy as np
import ml_dtypes

N_NODES = 50000
N_EDGES = 312500
WIDTH = 256
N_CORES = 8
P = 128
N_BINS = 49
NODES_PER_CORE = N_NODES // N_CORES  # 6250
DST_SLOTS = N_BINS * P  # 6272 padded dst slots per core
LN_EPS = 1e-5
TABLE_ROWS = 32768


def _shard_nodes(deg_in):
    """Assign each node to a core (exactly NODES_PER_CORE each), balancing
    total edge counts.  Greedy over nodes sorted by in-degree."""
    import heapq

    order = np.argsort(-deg_in, kind="stable")
    node_core = np.empty(N_NODES, np.int32)
    cnt = np.zeros(N_CORES, np.int64)
    heap = [(0, c) for c in range(N_CORES)]
    heapq.heapify(heap)
    for n in order:
        while True:
            e, c = heapq.heappop(heap)
            if cnt[c] < NODES_PER_CORE:
                break
        node_core[n] = c
        cnt[c] += 1
        if cnt[c] < NODES_PER_CORE:
            heapq.heappush(heap, (e + int(deg_in[n]), c))
    return node_core


def _pack_bins(nodes, degs, targets):
    """Greedy bin-pack `nodes` (with edge counts `degs`) into len(targets)
    bins of <=P nodes and ~targets[b] edges.  Returns bin id per node."""
    nb = len(targets)
    rem_e = np.asarray(targets, np.int64).copy()
    rem_s = np.full(nb, P, np.int64)
    order = np.argsort(-degs, kind="stable")
    bin_of = np.empty(len(nodes), np.int32)
    for i in order:
        d = degs[i]
        open_b = np.flatnonzero(rem_s > 0)
        fits = open_b[rem_e[open_b] >= d]
        b = (fits if len(fits) else open_b)[
            np.argmax(rem_e[fits if len(fits) else open_b])
        ]
        bin_of[i] = b
        rem_e[b] -= d
        rem_s[b] -= 1
    return bin_of


def _preprocess(edge_index, x, dinv):
    """Returns (profile, per-core input dict pieces, out node mapping)."""
    src = np.asarray(edge_index[0]).astype(np.int64)
    dst = np.asarray(edge_index[1]).astype(np.int64)
    deg_in = np.bincount(dst, minlength=N_NODES)

    node_core = _shard_nodes(deg_in)
    core_edges = np.bincount(node_core[dst], minlength=N_CORES)
    m = int(core_edges.max())
    tt = math.ceil(m / P) + 2
    n7 = tt - 6 * N_BINS
    assert 0 < n7 <= N_BINS, (tt, n7)
    targets = np.array([7 * P] * n7 + [6 * P] * (N_BINS - n7), np.int64)

    cores = []
    node_slot = np.empty(N_NODES, np.int64)  # slot (0..6271) within its core
    for c in range(N_CORES):
        nodes = np.flatnonzero(node_core == c)
        degs = deg_in[nodes]
        bin_of = _pack_bins(nodes, degs, targets)
        cnt = np.bincount(bin_of, weights=degs, minlength=N_BINS).astype(np.int64)
        # order bins by edge count desc so the per-position tile profile is
        # aligned across cores
        border = np.argsort(-cnt, kind="stable")
        rank = np.empty(N_BINS, np.int64)
        rank[border] = np.arange(N_BINS)
        bin_of = rank[bin_of]
        cnt = cnt[border]
        # column within bin
        order2 = np.argsort(bin_of, kind="stable")
        col = np.empty(len(nodes), np.int64)
        col[order2] = np.arange(len(nodes)) - np.concatenate(
            [[0], np.cumsum(np.bincount(bin_of, minlength=N_BINS))]
        )[bin_of[order2]]
        assert col.max() < P
        node_slot[nodes] = bin_of * P + col
        cores.append((nodes, bin_of, col, cnt))

    prof = np.zeros(N_BINS, np.int64)
    for _, _, _, cnt in cores:
        prof = np.maximum(prof, np.ceil(cnt / P).astype(np.int64))
    profile = tuple(int(t) for t in prof)
    ttot = int(sum(profile))
    toff = np.concatenate([[0], np.cumsum(prof)])

    xd = (np.asarray(x, np.float64) * dinv[:, None]).astype(np.float32)

    in_maps = []
    for c in range(N_CORES):
        nodes, bin_of, col, cnt = cores[c]
        e_mask = node_core[dst] == c
        e_src = src[e_mask]
        e_dst = dst[e_mask]
        e_bin = node_slot[e_dst] // P
        e_col = node_slot[e_dst] % P
        eorder = np.argsort(e_bin, kind="stable")
        e_src, e_bin, e_col = e_src[eorder], e_bin[eorder], e_col[eorder]
        # position within bin -> (tile, pos)
        within = np.arange(len(e_src)) - np.concatenate(
            [[0], np.cumsum(np.bincount(e_bin, minlength=N_BINS))]
        )[e_bin]
        tile = toff[e_bin] + within // P
        pos = within % P

        uniq, inv = np.unique(e_src, return_inverse=True)
        assert len(uniq) < TABLE_ROWS, len(uniq)
        table = np.zeros((TABLE_ROWS, WIDTH), ml_dtypes.bfloat16)
        table[: len(uniq)] = xd[uniq]

        idx_flat = np.zeros(ttot * P, np.int16)
        idx_flat[tile * P + pos] = inv.astype(np.int16)
        a = idx_flat.reshape(ttot * 8, 16).transpose(1, 0)
        idx = np.ascontiguousarray(np.tile(a, (8, 1)))

        s_all = np.zeros((P, ttot * P), ml_dtypes.float8_e4m3)
        s_all[pos, tile * P + e_col] = 1.0

        # per-slot node map, dinv tables, self-feature transpose
        slot_node = np.full(DST_SLOTS, -1, np.int64)
        slot_node[bin_of * P + col] = nodes
        valid = slot_node >= 0
        dinvtab = np.ones((P, N_BINS), np.float32)
        dv = np.ones(DST_SLOTS, np.float32)
        dv[valid] = dinv[slot_node[valid]].astype(np.float32)
        dinvtab = dv.reshape(N_BINS, P).transpose(1, 0).copy()
        dinvW = dinvtab / WIDTH
        xs = np.zeros((DST_SLOTS, WIDTH), np.float32)
        xs[valid] = xd[slot_node[valid]]
        xtself = np.concatenate([xs[:, :P].T, xs[:, P:].T], axis=1).astype(
            ml_dtypes.bfloat16
        )
        in_maps.append(
            {
                "tab": table,
                "idx": idx,
                "sall": np.ascontiguousarray(s_all),
                "xtself": np.ascontiguousarray(xtself),
                "dcon": np.ascontiguousarray(
                    np.concatenate([dinvtab, dinvW], axis=1)
                ),
                "_slot_node": slot_node,
            }
        )
    return profile, in_maps


def _build_program(profile, bias_mean, generic_affine):
    import concourse.bass as bass
    import concourse.tile as tile
    from concourse import bacc as bacc_mod
    from concourse import mybir
    from contextlib import ExitStack

    f32 = mybir.dt.float32
    bf16 = mybir.dt.bfloat16
    fp8 = mybir.dt.float8e4
    i16 = mybir.dt.int16
    Alu = mybir.AluOpType
    Act = mybir.ActivationFunctionType
    ttot = int(sum(profile))
    toff = np.concatenate([[0], np.cumsum(profile)]).astype(np.int64)

    # fcon (f32) columns: [bias (W) | gamma? | beta?]
    FW = WIDTH + (2 * WIDTH if generic_affine else 0)

    nc = bacc_mod.Bacc(None, target_bir_lowering=False, debug=False, num_swdge_queues=4)
    tab_d = nc.declare_dram_parameter("tab", [TABLE_ROWS, WIDTH], bf16, isOutput=False)
    idx_d = nc.declare_dram_parameter("idx", [P, 8 * ttot], i16, isOutput=False)
    sall_d = nc.declare_dram_parameter("sall", [P, ttot * P], fp8, isOutput=False)
    xts_d = nc.declare_dram_parameter("xtself", [P, 2 * DST_SLOTS], bf16, isOutput=False)
    dcon_d = nc.declare_dram_parameter("dcon", [P, 2 * N_BINS], f32, isOutput=False)
    wt_d = nc.declare_dram_parameter("wt", [P, 2 * (WIDTH + 1)], bf16, isOutput=False)
    fcon_d = nc.declare_dram_parameter("fcon", [P, FW], f32, isOutput=False)
    out_d = nc.declare_dram_parameter("out", [DST_SLOTS, WIDTH], f32, isOutput=True)

    with tile.TileContext(nc) as tc:
        with ExitStack() as ctx:
            const = ctx.enter_context(tc.tile_pool(name="const", bufs=1))
            gpool = ctx.enter_context(tc.tile_pool(name="g", bufs=10))
            spool = ctx.enter_context(tc.tile_pool(name="s", bufs=10))
            apool = ctx.enter_context(tc.tile_pool(name="aggT", bufs=2))
            ypool = ctx.enter_context(tc.tile_pool(name="y", bufs=3))
            stat = ctx.enter_context(tc.tile_pool(name="stat", bufs=4))
            ppool = ctx.enter_context(tc.tile_pool(name="psA", bufs=2, space="PSUM"))
            opsum = ctx.enter_context(tc.tile_pool(name="psO", bufs=2, space="PSUM"))

            idx_sb = const.tile([P, 8 * ttot], i16)
            nc.sync.dma_start(idx_sb[:], idx_d[:, :])
            wt_sb = const.tile([P, 2 * (WIDTH + 1)], bf16)
            nc.scalar.dma_start(wt_sb[:], wt_d[:, :])
            fcon_sb = const.tile([P, FW], f32)
            nc.scalar.dma_start(fcon_sb[:], fcon_d[:, :])
            dcon_sb = const.tile([P, 2 * N_BINS], f32)
            nc.scalar.dma_start(dcon_sb[:], dcon_d[:, :])
            xts_sb = const.tile([P, 2 * DST_SLOTS], bf16)
            nc.scalar.dma_start(xts_sb[:], xts_d[:, :])
            eps_sb = const.tile([P, 1], f32)
            nc.vector.memset(eps_sb[:], LN_EPS)

            bias_sb = fcon_sb[:, :WIDTH]
            if generic_affine:
                gamma_sb = fcon_sb[:, WIDTH : 2 * WIDTH]
                beta_sb = fcon_sb[:, 2 * WIDTH : 3 * WIDTH]

            for b in range(N_BINS):
                nt = int(profile[b])
                t0 = int(toff[b])
                g = gpool.tile([P, nt, WIDTH], bf16, tag="g")
                nc.gpsimd.dma_gather(
                    g[:],
                    tab_d[:, :],
                    idx_sb[:, 8 * t0 : 8 * (t0 + nt)],
                    nt * P,
                    nt * P,
                    WIDTH,
                    queue_num=b % 4,
                )
                s = spool.tile([P, nt * P], fp8, tag="s")
                nc.scalar.dma_start(s[:], sall_d[:, t0 * P : (t0 + nt) * P])

                ps0 = ppool.tile([P, P], f32, tag="ps0")
                ps1 = ppool.tile([P, P], f32, tag="ps1")
                for k in range(nt):
                    nc.tensor.matmul(
                        out=ps0[:],
                        lhsT=g[:, k, 0:P],
                        rhs=s[:, k * P : (k + 1) * P],
                        start=(k == 0),
                        stop=(k == nt - 1),
                    )
                    nc.tensor.matmul(
                        out=ps1[:],
                        lhsT=g[:, k, P:WIDTH],
                        rhs=s[:, k * P : (k + 1) * P],
                        start=(k == 0),
                        stop=(k == nt - 1),
                    )
                a0 = apool.tile([P, P], bf16, tag="a0")
                nc.scalar.copy(a0[:], ps0[:])
                a1 = apool.tile([P, P], bf16, tag="a1")
                nc.scalar.copy(a1[:], ps1[:])
                po = opsum.tile([P, WIDTH + 1], f32, tag="po")
                nc.tensor.matmul(
                    out=po[:], lhsT=a0[:], rhs=wt_sb[:, : WIDTH + 1],
                    start=True, stop=False,
                )
                nc.tensor.matmul(
                    out=po[:], lhsT=a1[:], rhs=wt_sb[:, WIDTH + 1 :],
                    start=False, stop=False,
                )
                nc.tensor.matmul(
                    out=po[:],
                    lhsT=xts_sb[:, b * P : (b + 1) * P],
                    rhs=wt_sb[:, : WIDTH + 1],
                    start=False, stop=False,
                )
                nc.tensor.matmul(
                    out=po[:],
                    lhsT=xts_sb[:, DST_SLOTS + b * P : DST_SLOTS + (b + 1) * P],
                    rhs=wt_sb[:, WIDTH + 1 :],
                    start=False, stop=True,
                )
                # ---- epilogue: y = po*dinv_d + bias; LayerNorm; ReLU ----
                ys = ypool.tile([P, WIDTH], f32, tag="ys")
                nc.vector.tensor_scalar(
                    out=ys[:], in0=po[:, :WIDTH],
                    scalar1=dcon_sb[:, b : b + 1], scalar2=None, op0=Alu.mult,
                )
                y = ypool.tile([P, WIDTH], f32, tag="y")
                nc.vector.tensor_tensor(out=y[:], in0=ys[:], in1=bias_sb, op=Alu.add)
                sq = ypool.tile([P, WIDTH], f32, tag="sq")
                ssq = stat.tile([P, 1], f32, tag="ssq")
                nc.scalar.activation(
                    out=sq[:], in_=y[:], func=Act.Square, accum_out=ssq[:]
                )
                mu = stat.tile([P, 1], f32, tag="mu")
                nc.vector.tensor_scalar(
                    out=mu[:],
                    in0=po[:, WIDTH : WIDTH + 1],
                    scalar1=dcon_sb[:, N_BINS + b : N_BINS + b + 1],
                    scalar2=float(bias_mean),
                    op0=Alu.mult,
                    op1=Alu.add,
                )
                m2 = stat.tile([P, 1], f32, tag="m2")
                nc.scalar.square(m2[:], mu[:])
                var = stat.tile([P, 1], f32, tag="var")
                nc.vector.tensor_scalar(
                    out=var[:],
                    in0=ssq[:],
                    scalar1=1.0 / WIDTH,
                    scalar2=m2[:, :1],
                    op0=Alu.mult,
                    op1=Alu.subtract,
                )
                sd = stat.tile([P, 1], f32, tag="sd")
                nc.scalar.activation(
                    out=sd[:], in_=var[:], func=Act.Sqrt, bias=eps_sb[:, :1]
                )
                rstd = stat.tile([P, 1], f32, tag="rstd")
                nc.vector.reciprocal(rstd[:], sd[:])
                # rm = -mu * rstd so that LN+ReLU is one ACT op:
                # relu(y*rstd + rm) = relu((y - mu) * rstd)
                rm = stat.tile([P, 1], f32, tag="rm")
                nc.vector.tensor_scalar(
                    out=rm[:],
                    in0=mu[:],
                    scalar1=rstd[:, :1],
                    scalar2=-1.0,
                    op0=Alu.mult,
                    op1=Alu.mult,
                )
                if generic_affine:
                    t1 = ypool.tile([P, WIDTH], f32, tag="t1")
                    nc.scalar.activation(
                        out=t1[:], in_=y[:], func=Act.Identity,
                        scale=rstd[:, :1], bias=rm[:, :1],
                    )
                    t2 = ypool.tile([P, WIDTH], f32, tag="t2")
                    nc.vector.tensor_tensor(out=t2[:], in0=t1[:], in1=gamma_sb, op=Alu.mult)
                    t3 = ypool.tile([P, WIDTH], f32, tag="t3")
                    nc.vector.tensor_tensor(out=t3[:], in0=t2[:], in1=beta_sb, op=Alu.add)
                    yo = ypool.tile([P, WIDTH], f32, tag="yo")
                    nc.scalar.activation(out=yo[:], in_=t3[:], func=Act.Relu)
                else:
                    yo = ypool.tile([P, WIDTH], f32, tag="yo")
                    nc.scalar.activation(
                        out=yo[:], in_=y[:], func=Act.Relu,
                        scale=rstd[:, :1], bias=rm[:, :1],
                    )
                nc.sync.dma_start(out_d[b * P : (b + 1) * P, :], yo[:])
    return nc


_PROGRAM_CACHE = {}
_PREP_CACHE = {}


def kernel(x, edge_index, W, b, gamma, beta, _run_kwargs=None):
    from concourse.bass_utils import run_bass_kernel_spmd

    x = np.asarray(x)
    W = np.asarray(W)
    bias = np.asarray(b, dtype=np.float64)
    gamma = np.asarray(gamma)
    beta = np.asarray(beta)

    ekey = hash(np.asarray(edge_index).tobytes()) ^ hash(x.tobytes())
    if ekey not in _PREP_CACHE:
        dst = np.asarray(edge_index[1]).astype(np.int64)
        deg = np.bincount(dst, minlength=N_NODES).astype(np.float64) + 1.0
        dinv = 1.0 / np.sqrt(deg)
        _PREP_CACHE.clear()
        _PREP_CACHE[ekey] = _preprocess(edge_index, x, dinv)
    profile, in_maps = _PREP_CACHE[ekey]

    generic_affine = not (np.all(gamma == 1.0) and np.all(beta == 0.0))
    bias_mean = float(bias.mean())
    key = (profile, generic_affine, bias_mean)
    if key not in _PROGRAM_CACHE:
        nc = _build_program(profile, bias_mean, generic_affine)
        nc.finalize()
        _PROGRAM_CACHE[key] = nc
    nc = _PROGRAM_CACHE[key]

    WT32 = W.T.astype(np.float32)
    rs = WT32.sum(axis=1, keepdims=True)
    WTe = np.concatenate([WT32, rs], axis=1).astype(ml_dtypes.bfloat16)
    wt = np.ascontiguousarray(np.concatenate([WTe[:P], WTe[P:]], axis=1))
    fparts = [np.tile(bias.astype(np.float32)[None, :], (P, 1))]
    if generic_affine:
        fparts.append(np.tile(gamma.astype(np.float32)[None, :], (P, 1)))
        fparts.append(np.tile(beta.astype(np.float32)[None, :], (P, 1)))
    fcon = np.ascontiguousarray(np.concatenate(fparts, axis=1, dtype=np.float32))

    run_maps = []
    for c in range(N_CORES):
        m = {k: v for k, v in in_maps[c].items() if not k.startswith("_")}
        m["wt"] = wt
        m["fcon"] = fcon
        run_maps.append(m)

    kwargs = dict(_run_kwargs or {})
    kwargs.pop("_result", None)
    rr = run_bass_kernel_spmd(nc, run_maps, list(range(N_CORES)), **kwargs)
    out = np.zeros((N_NODES, WIDTH), np.float32)
    for c in range(N_CORES):
        slot_node = in_maps[c]["_slot_node"]
        valid = slot_node >= 0
        out[slot_node[valid]] = rr.results[c]["out"][valid]
    if _run_kwargs is not None:
        _run_kwargs["_result"] = rr
    return out


# revision 13
# speedup vs baseline: 2.0730x; 1.1440x over previous
"""GCN block (GCNConv + LayerNorm + ReLU) on 8 Trainium2 NeuronCores.

Strategy v3 (gather-descriptor-rate aware):
  - out = LN((A_norm @ x) @ W^T + b): aggregate raw features first, so the
    random gather runs on node-major x.
  - The SWDGE gather is descriptor-generation-bound (~3 ns/row across the
    4 ucode queues), so the kernel minimizes gathered rows:
      * self-loops are NOT gathered: their diag(dinv^2) x W^T term is a dense
        matmul against an SBUF-resident transposed shard copy;
      * destination nodes are bin-packed (host side) into 49 blocks per core
        with edge counts just under a multiple of 128, so tile padding is ~1%;
      * per-core gather tables hold only the ~27k unique source rows a core
        needs, so int16 indices cover them without an even/odd table split.
  - norm factorizes: dinv[src] is folded into the gather table rows on the
    host; dinv[dst] rides inside the shipped fp8 scatter matrices S
    (S[e, dstcol] = dinv_dst; fp8 quantization of the row scale cancels in
    LayerNorm, and the self term is made scale-consistent on the host).
  - bias enters PSUM via a rank-1 (K=1) matmul whose extra row-sum column
    (from WTe) also makes the LN mean free.  Epilogue is then just
    Square-accum / small stats / one fused scale+bias+ReLU ACT op.
"""

import math
import sys

sys.path.insert(0, "/opt/trn_rl_repo")

import numpy as np
import ml_dtypes

N_NODES = 50000
N_EDGES = 312500
WIDTH = 256
N_CORES = 8
P = 128
N_BINS = 49
SGROUP = 4  # bins per S-matrix DMA batch
NODES_PER_CORE = N_NODES // N_CORES  # 6250
DST_SLOTS = N_BINS * P  # 6272 padded dst slots per core
LN_EPS = 1e-5
TABLE_ROWS = 32768


def _shard_nodes(deg_in):
    """Assign each node to a core (exactly NODES_PER_CORE each), balancing
    total edge counts.  Greedy over nodes sorted by in-degree."""
    import heapq

    order = np.argsort(-deg_in, kind="stable")
    node_core = np.empty(N_NODES, np.int32)
    cnt = np.zeros(N_CORES, np.int64)
    heap = [(0, c) for c in range(N_CORES)]
    heapq.heapify(heap)
    for n in order:
        while True:
            e, c = heapq.heappop(heap)
            if cnt[c] < NODES_PER_CORE:
                break
        node_core[n] = c
        cnt[c] += 1
        if cnt[c] < NODES_PER_CORE:
            heapq.heappush(heap, (e + int(deg_in[n]), c))
    return node_core


def _pack_bins(nodes, degs, targets):
    """Greedy bin-pack `nodes` (with edge counts `degs`) into len(targets)
    bins of <=P nodes and ~targets[b] edges.  Returns bin id per node."""
    nb = len(targets)
    rem_e = np.asarray(targets, np.int64).copy()
    rem_s = np.full(nb, P, np.int64)
    order = np.argsort(-degs, kind="stable")
    bin_of = np.empty(len(nodes), np.int32)
    for i in order:
        d = degs[i]
        open_b = np.flatnonzero(rem_s > 0)
        fits = open_b[rem_e[open_b] >= d]
        b = (fits if len(fits) else open_b)[
            np.argmax(rem_e[fits if len(fits) else open_b])
        ]
        bin_of[i] = b
        rem_e[b] -= d
        rem_s[b] -= 1
    return bin_of


def _preprocess(edge_index, x, dinv):
    src = np.asarray(edge_index[0]).astype(np.int64)
    dst = np.asarray(edge_index[1]).astype(np.int64)
    deg_in = np.bincount(dst, minlength=N_NODES)

    node_core = _shard_nodes(deg_in)
    core_edges = np.bincount(node_core[dst], minlength=N_CORES)
    m = int(core_edges.max())
    tt = math.ceil(m / P) + 2
    n7 = tt - 6 * N_BINS
    assert 0 < n7 <= N_BINS, (tt, n7)
    targets = np.array([7 * P] * n7 + [6 * P] * (N_BINS - n7), np.int64)

    cores = []
    node_slot = np.empty(N_NODES, np.int64)  # slot (0..6271) within its core
    for c in range(N_CORES):
        nodes = np.flatnonzero(node_core == c)
        degs = deg_in[nodes]
        bin_of = _pack_bins(nodes, degs, targets)
        cnt = np.bincount(bin_of, weights=degs, minlength=N_BINS).astype(np.int64)
        border = np.argsort(-cnt, kind="stable")
        rank = np.empty(N_BINS, np.int64)
        rank[border] = np.arange(N_BINS)
        bin_of = rank[bin_of]
        cnt = cnt[border]
        order2 = np.argsort(bin_of, kind="stable")
        col = np.empty(len(nodes), np.int64)
        col[order2] = np.arange(len(nodes)) - np.concatenate(
            [[0], np.cumsum(np.bincount(bin_of, minlength=N_BINS))]
        )[bin_of[order2]]
        assert col.max() < P
        node_slot[nodes] = bin_of * P + col
        cores.append((nodes, bin_of, col, cnt))

    prof = np.zeros(N_BINS, np.int64)
    for _, _, _, cnt in cores:
        prof = np.maximum(prof, np.ceil(cnt / P).astype(np.int64))
    profile = tuple(int(t) for t in prof)
    ttot = int(sum(profile))
    toff = np.concatenate([[0], np.cumsum(prof)])

    xd = (np.asarray(x, np.float64) * dinv[:, None]).astype(np.float32)
    # fp8-quantized destination scales (must match what the fp8 S delivers)
    dq8 = dinv.astype(np.float32).astype(ml_dtypes.float8_e4m3).astype(np.float32)

    in_maps = []
    for c in range(N_CORES):
        nodes, bin_of, col, cnt = cores[c]
        e_mask = node_core[dst] == c
        e_src = src[e_mask]
        e_dst = dst[e_mask]
        e_bin = node_slot[e_dst] // P
        e_col = node_slot[e_dst] % P
        eorder = np.argsort(e_bin, kind="stable")
        e_src, e_dst, e_bin, e_col = (
            e_src[eorder], e_dst[eorder], e_bin[eorder], e_col[eorder],
        )
        within = np.arange(len(e_src)) - np.concatenate(
            [[0], np.cumsum(np.bincount(e_bin, minlength=N_BINS))]
        )[e_bin]
        tile = toff[e_bin] + within // P
        pos = within % P

        uniq, inv = np.unique(e_src, return_inverse=True)
        assert len(uniq) < TABLE_ROWS, len(uniq)
        table = np.zeros((TABLE_ROWS, WIDTH), ml_dtypes.bfloat16)
        table[: len(uniq)] = xd[uniq]

        idx_flat = np.zeros(ttot * P, np.int16)
        idx_flat[tile * P + pos] = inv.astype(np.int16)
        a = idx_flat.reshape(ttot * 8, 16).transpose(1, 0)
        idx = np.ascontiguousarray(np.tile(a, (8, 1)))

        s_all = np.zeros((P, ttot * P), ml_dtypes.float8_e4m3)
        s_all[pos, tile * P + e_col] = dinv[e_dst].astype(np.float32)

        slot_node = np.full(DST_SLOTS, -1, np.int64)
        slot_node[bin_of * P + col] = nodes
        valid = slot_node >= 0
        xs = np.zeros((DST_SLOTS, WIDTH), np.float32)
        vn = slot_node[valid]
        # xd already carries one dinv factor; the extra dq8 factor matches
        # the fp8-quantized dst scale the edge path gets through S.
        xs[valid] = xd[vn] * dq8[vn][:, None]
        xtself = np.concatenate([xs[:, :P].T, xs[:, P:].T], axis=1).astype(
            ml_dtypes.bfloat16
        )
        in_maps.append(
            {
                "tab": table,
                "idx": idx,
                "sall": np.ascontiguousarray(s_all),
                "xtself": np.ascontiguousarray(xtself),
                "_slot_node": slot_node,
            }
        )
    return profile, in_maps


def _build_program(profile, generic_affine):
    import concourse.bass as bass
    import concourse.tile as tile
    from concourse import bacc as bacc_mod
    from concourse import mybir
    from contextlib import ExitStack

    f32 = mybir.dt.float32
    bf16 = mybir.dt.bfloat16
    fp8 = mybir.dt.float8e4
    i16 = mybir.dt.int16
    Alu = mybir.AluOpType
    Act = mybir.ActivationFunctionType
    ttot = int(sum(profile))
    toff = np.concatenate([[0], np.cumsum(profile)]).astype(np.int64)
    HEAD_BINS = 4
    thead = int(toff[HEAD_BINS])
    n_groups = math.ceil(N_BINS / SGROUP)

    nc = bacc_mod.Bacc(None, target_bir_lowering=False, debug=False, num_swdge_queues=4)
    tab_d = nc.declare_dram_parameter("tab", [TABLE_ROWS, WIDTH], bf16, isOutput=False)
    idx_d = nc.declare_dram_parameter("idx", [P, 8 * ttot], i16, isOutput=False)
    sall_d = nc.declare_dram_parameter("sall", [P, ttot * P], fp8, isOutput=False)
    xts_d = nc.declare_dram_parameter("xtself", [P, 2 * DST_SLOTS], bf16, isOutput=False)
    wt_d = nc.declare_dram_parameter("wt", [P, 2 * (WIDTH + 1)], bf16, isOutput=False)
    bcon_d = nc.declare_dram_parameter("bcon", [1, WIDTH + 1], bf16, isOutput=False)
    if generic_affine:
        fcon_d = nc.declare_dram_parameter("fcon", [P, 2 * WIDTH], f32, isOutput=False)
    out_d = nc.declare_dram_parameter("out", [DST_SLOTS, WIDTH], bf16, isOutput=True)

    with tile.TileContext(nc) as tc:
        with ExitStack() as ctx:
            const = ctx.enter_context(tc.tile_pool(name="const", bufs=1))
            gpool = ctx.enter_context(tc.tile_pool(name="g", bufs=10))
            spool = ctx.enter_context(tc.tile_pool(name="s", bufs=4))
            apool = ctx.enter_context(tc.tile_pool(name="aggT", bufs=2))
            ypool = ctx.enter_context(tc.tile_pool(name="y", bufs=3))
            stat = ctx.enter_context(tc.tile_pool(name="stat", bufs=4))
            ppool = ctx.enter_context(tc.tile_pool(name="psA", bufs=2, space="PSUM"))
            opsum = ctx.enter_context(tc.tile_pool(name="psO", bufs=2, space="PSUM"))

            idx_a = const.tile([P, 8 * thead], i16)
            nc.sync.dma_start(idx_a[:], idx_d[:, : 8 * thead])
            idx_b = const.tile([P, 8 * (ttot - thead)], i16)
            nc.sync.dma_start(idx_b[:], idx_d[:, 8 * thead :])
            wt_sb = const.tile([P, 2 * (WIDTH + 1)], bf16)
            nc.scalar.dma_start(wt_sb[:], wt_d[:, :])
            bcon_sb = const.tile([1, WIDTH + 1], bf16)
            nc.scalar.dma_start(bcon_sb[:], bcon_d[:, :])
            ones_sb = const.tile([1, P], bf16)
            nc.vector.memset(ones_sb[:], 1.0)
            eps_sb = const.tile([P, 1], f32)
            nc.vector.memset(eps_sb[:], LN_EPS)
            if generic_affine:
                fcon_sb = const.tile([P, 2 * WIDTH], f32)
                nc.scalar.dma_start(fcon_sb[:], fcon_d[:, :])
                gamma_sb = fcon_sb[:, :WIDTH]
                beta_sb = fcon_sb[:, WIDTH : 2 * WIDTH]
            xts_sb = const.tile([P, 2 * DST_SLOTS], bf16)
            nc.scalar.dma_start(xts_sb[:], xts_d[:, :])

            sgroups = [None] * n_groups

            for b in range(N_BINS):
                nt = int(profile[b])
                t0 = int(toff[b])
                g = gpool.tile([P, nt, WIDTH], bf16, tag="g")
                if t0 + nt <= thead:
                    isb, ioff = idx_a, t0
                else:
                    isb, ioff = idx_b, t0 - thead
                nc.gpsimd.dma_gather(
                    g[:],
                    tab_d[:, :],
                    isb[:, 8 * ioff : 8 * (ioff + nt)],
                    nt * P,
                    nt * P,
                    WIDTH,
                    queue_num=b % 4,
                )
                gb = b // SGROUP
                if sgroups[gb] is None:
                    b0 = gb * SGROUP
                    b1 = min(b0 + SGROUP, N_BINS)
                    gt0, gt1 = int(toff[b0]), int(toff[b1])
                    s = spool.tile([P, (gt1 - gt0) * P], fp8, tag="s")
                    nc.scalar.dma_start(s[:], sall_d[:, gt0 * P : gt1 * P])
                    sgroups[gb] = (s, gt0)
                s, gt0 = sgroups[gb]
                soff = (t0 - gt0) * P

                ps0 = ppool.tile([P, P], f32, tag="ps0")
                ps1 = ppool.tile([P, P], f32, tag="ps1")
                for k in range(nt):
                    nc.tensor.matmul(
                        out=ps0[:],
                        lhsT=g[:, k, 0:P],
                        rhs=s[:, soff + k * P : soff + (k + 1) * P],
                        start=(k == 0),
                        stop=(k == nt - 1),
                    )
                    nc.tensor.matmul(
                        out=ps1[:],
                        lhsT=g[:, k, P:WIDTH],
                        rhs=s[:, soff + k * P : soff + (k + 1) * P],
                        start=(k == 0),
                        stop=(k == nt - 1),
                    )
                a0 = apool.tile([P, P], bf16, tag="a0")
                nc.vector.tensor_scalar(
                    out=a0[:], in0=ps0[:], scalar1=1.0, scalar2=None, op0=Alu.mult
                )
                a1 = apool.tile([P, P], bf16, tag="a1")
                nc.vector.tensor_scalar(
                    out=a1[:], in0=ps1[:], scalar1=1.0, scalar2=None, op0=Alu.mult
                )
                po = opsum.tile([P, WIDTH + 1], f32, tag="po")
                nc.tensor.matmul(
                    out=po[:], lhsT=ones_sb[:], rhs=bcon_sb[:],
                    start=True, stop=False,
                )
                nc.tensor.matmul(
                    out=po[:], lhsT=a0[:], rhs=wt_sb[:, : WIDTH + 1],
                    start=False, stop=False,
                )
                nc.tensor.matmul(
                    out=po[:], lhsT=a1[:], rhs=wt_sb[:, WIDTH + 1 :],
                    start=False, stop=False,
                )
                nc.tensor.matmul(
                    out=po[:],
                    lhsT=xts_sb[:, b * P : (b + 1) * P],
                    rhs=wt_sb[:, : WIDTH + 1],
                    start=False, stop=False,
                )
                nc.tensor.matmul(
                    out=po[:],
                    lhsT=xts_sb[:, DST_SLOTS + b * P : DST_SLOTS + (b + 1) * P],
                    rhs=wt_sb[:, WIDTH + 1 :],
                    start=False, stop=True,
                )
                # ---- epilogue: LayerNorm(po) + ReLU (bias already in po) ----
                sq = ypool.tile([P, WIDTH], f32, tag="sq")
                ssq = stat.tile([P, 1], f32, tag="ssq")
                nc.scalar.activation(
                    out=sq[:], in_=po[:, :WIDTH], func=Act.Square, accum_out=ssq[:]
                )
                mu = stat.tile([P, 1], f32, tag="mu")
                nc.vector.tensor_scalar(
                    out=mu[:],
                    in0=po[:, WIDTH : WIDTH + 1],
                    scalar1=1.0 / WIDTH,
                    scalar2=None,
                    op0=Alu.mult,
                )
                m2 = stat.tile([P, 1], f32, tag="m2")
                nc.scalar.square(m2[:], mu[:])
                var = stat.tile([P, 1], f32, tag="var")
                nc.vector.tensor_scalar(
                    out=var[:],
                    in0=ssq[:],
                    scalar1=1.0 / WIDTH,
                    scalar2=m2[:, :1],
                    op0=Alu.mult,
                    op1=Alu.subtract,
                )
                sd = stat.tile([P, 1], f32, tag="sd")
                nc.scalar.activation(
                    out=sd[:], in_=var[:], func=Act.Sqrt, bias=eps_sb[:, :1]
                )
                rstd = stat.tile([P, 1], f32, tag="rstd")
                nc.vector.reciprocal(rstd[:], sd[:])
                rm = stat.tile([P, 1], f32, tag="rm")
                nc.vector.tensor_scalar(
                    out=rm[:],
                    in0=mu[:],
                    scalar1=rstd[:, :1],
                    scalar2=-1.0,
                    op0=Alu.mult,
                    op1=Alu.mult,
                )
                if generic_affine:
                    t1 = ypool.tile([P, WIDTH], f32, tag="t1")
                    nc.scalar.activation(
                        out=t1[:], in_=po[:, :WIDTH], func=Act.Identity,
                        scale=rstd[:, :1], bias=rm[:, :1],
                    )
                    t2 = ypool.tile([P, WIDTH], f32, tag="t2")
                    nc.vector.tensor_tensor(out=t2[:], in0=t1[:], in1=gamma_sb, op=Alu.mult)
                    t3 = ypool.tile([P, WIDTH], f32, tag="t3")
                    nc.vector.tensor_tensor(out=t3[:], in0=t2[:], in1=beta_sb, op=Alu.add)
                    yo = ypool.tile([P, WIDTH], bf16, tag="yo")
                    nc.scalar.activation(out=yo[:], in_=t3[:], func=Act.Relu)
                else:
                    yo = ypool.tile([P, WIDTH], bf16, tag="yo")
                    nc.scalar.activation(
                        out=yo[:], in_=po[:, :WIDTH], func=Act.Relu,
                        scale=rstd[:, :1], bias=rm[:, :1],
                    )
                nc.sync.dma_start(out_d[b * P : (b + 1) * P, :], yo[:])
    return nc


_PROGRAM_CACHE = {}
_PREP_CACHE = {}


def kernel(x, edge_index, W, b, gamma, beta, _run_kwargs=None):
    from concourse.bass_utils import run_bass_kernel_spmd

    x = np.asarray(x)
    W = np.asarray(W)
    bias = np.asarray(b, dtype=np.float64)
    gamma = np.asarray(gamma)
    beta = np.asarray(beta)

    ekey = hash(np.asarray(edge_index).tobytes()) ^ hash(x.tobytes())
    if ekey not in _PREP_CACHE:
        dst = np.asarray(edge_index[1]).astype(np.int64)
        deg = np.bincount(dst, minlength=N_NODES).astype(np.float64) + 1.0
        dinv = 1.0 / np.sqrt(deg)
        _PREP_CACHE.clear()
        _PREP_CACHE[ekey] = _preprocess(edge_index, x, dinv)
    profile, in_maps = _PREP_CACHE[ekey]

    generic_affine = not (np.all(gamma == 1.0) and np.all(beta == 0.0))
    key = (profile, generic_affine)
    if key not in _PROGRAM_CACHE:
        nc = _build_program(profile, generic_affine)
        nc.finalize()
        _PROGRAM_CACHE[key] = nc
    nc = _PROGRAM_CACHE[key]

    WT32 = W.T.astype(np.float32)
    rs = WT32.sum(axis=1, keepdims=True)
    WTe = np.concatenate([WT32, rs], axis=1).astype(ml_dtypes.bfloat16)
    wt = np.ascontiguousarray(np.concatenate([WTe[:P], WTe[P:]], axis=1))
    bcon = np.concatenate([bias, [bias.sum()]]).astype(ml_dtypes.bfloat16)[None, :]
    bcon = np.ascontiguousarray(bcon)

    run_maps = []
    for c in range(N_CORES):
        m = {k: v for k, v in in_maps[c].items() if not k.startswith("_")}
        m["wt"] = wt
        m["bcon"] = bcon
        if generic_affine:
            m["fcon"] = np.ascontiguousarray(
                np.concatenate(
                    [
                        np.tile(gamma.astype(np.float32)[None, :], (P, 1)),
                        np.tile(beta.astype(np.float32)[None, :], (P, 1)),
                    ],
                    axis=1,
                )
            )
        run_maps.append(m)

    kwargs = dict(_run_kwargs or {})
    kwargs.pop("_result", None)
    rr = run_bass_kernel_spmd(nc, run_maps, list(range(N_CORES)), **kwargs)
    out = np.zeros((N_NODES, WIDTH), np.float32)
    for c in range(N_CORES):
        slot_node = in_maps[c]["_slot_node"]
        valid = slot_node >= 0
        out[slot_node[valid]] = rr.results[c]["out"][valid].astype(np.float32)
    if _run_kwargs is not None:
        _run_kwargs["_result"] = rr
    return out


# revision 19
# speedup vs baseline: 2.1731x; 1.0483x over previous
"""GCN block (GCNConv + LayerNorm + ReLU) on 8 Trainium2 NeuronCores.

Strategy v3 (gather-descriptor-rate aware):
  - out = LN((A_norm @ x) @ W^T + b): aggregate raw features first, so the
    random gather runs on node-major x.
  - The SWDGE gather is descriptor-generation-bound (~3 ns/row across the
    4 ucode queues), so the kernel minimizes gathered rows:
      * self-loops are NOT gathered: their diag(dinv^2) x W^T term is a dense
        matmul against an SBUF-resident transposed shard copy;
      * destination nodes are bin-packed (host side) into 49 blocks per core
        with edge counts just under a multiple of 128, so tile padding is ~1%;
      * per-core gather tables hold only the ~27k unique source rows a core
        needs, so int16 indices cover them without an even/odd table split.
  - norm factorizes: dinv[src] is folded into the gather table rows on the
    host; dinv[dst] rides inside the shipped fp8 scatter matrices S
    (S[e, dstcol] = dinv_dst; fp8 quantization of the row scale cancels in
    LayerNorm, and the self term is made scale-consistent on the host).
  - bias enters PSUM via a rank-1 (K=1) matmul whose extra row-sum column
    (from WTe) also makes the LN mean free.  Epilogue is then just
    Square-accum / small stats / one fused scale+bias+ReLU ACT op.
"""

import math
import sys

sys.path.insert(0, "/opt/trn_rl_repo")

import numpy as np
import ml_dtypes

N_NODES = 50000
N_EDGES = 312500
WIDTH = 256
N_CORES = 8
P = 128
N_BINS = 49
SGROUP = 4  # bins per S-matrix DMA batch
NODES_PER_CORE = N_NODES // N_CORES  # 6250
DST_SLOTS = N_BINS * P  # 6272 padded dst slots per core
LN_EPS = 1e-5
TABLE_ROWS = 32768


def _shard_nodes(deg_in):
    """Assign each node to a core (exactly NODES_PER_CORE each), balancing
    total edge counts.  Greedy over nodes sorted by in-degree."""
    import heapq

    order = np.argsort(-deg_in, kind="stable")
    node_core = np.empty(N_NODES, np.int32)
    cnt = np.zeros(N_CORES, np.int64)
    heap = [(0, c) for c in range(N_CORES)]
    heapq.heapify(heap)
    for n in order:
        while True:
            e, c = heapq.heappop(heap)
            if cnt[c] < NODES_PER_CORE:
                break
        node_core[n] = c
        cnt[c] += 1
        if cnt[c] < NODES_PER_CORE:
            heapq.heappush(heap, (e + int(deg_in[n]), c))
    return node_core


def _pack_bins(nodes, degs, targets):
    """Greedy bin-pack `nodes` (with edge counts `degs`) into len(targets)
    bins of <=P nodes and ~targets[b] edges.  Returns bin id per node."""
    nb = len(targets)
    rem_e = np.asarray(targets, np.int64).copy()
    rem_s = np.full(nb, P, np.int64)
    order = np.argsort(-degs, kind="stable")
    bin_of = np.empty(len(nodes), np.int32)
    for i in order:
        d = degs[i]
        open_b = np.flatnonzero(rem_s > 0)
        fits = open_b[rem_e[open_b] >= d]
        b = (fits if len(fits) else open_b)[
            np.argmax(rem_e[fits if len(fits) else open_b])
        ]
        bin_of[i] = b
        rem_e[b] -= d
        rem_s[b] -= 1
    return bin_of


def _preprocess(edge_index, x, dinv):
    src = np.asarray(edge_index[0]).astype(np.int64)
    dst = np.asarray(edge_index[1]).astype(np.int64)
    deg_in = np.bincount(dst, minlength=N_NODES)

    node_core = _shard_nodes(deg_in)
    core_edges = np.bincount(node_core[dst], minlength=N_CORES)
    m = int(core_edges.max())
    tt = math.ceil(m / P) + 2
    n7 = tt - 6 * N_BINS
    assert 0 < n7 <= N_BINS, (tt, n7)
    targets = np.array([7 * P] * n7 + [6 * P] * (N_BINS - n7), np.int64)

    cores = []
    node_slot = np.empty(N_NODES, np.int64)  # slot (0..6271) within its core
    for c in range(N_CORES):
        nodes = np.flatnonzero(node_core == c)
        degs = deg_in[nodes]
        bin_of = _pack_bins(nodes, degs, targets)
        cnt = np.bincount(bin_of, weights=degs, minlength=N_BINS).astype(np.int64)
        border = np.argsort(-cnt, kind="stable")
        rank = np.empty(N_BINS, np.int64)
        rank[border] = np.arange(N_BINS)
        bin_of = rank[bin_of]
        cnt = cnt[border]
        order2 = np.argsort(bin_of, kind="stable")
        col = np.empty(len(nodes), np.int64)
        col[order2] = np.arange(len(nodes)) - np.concatenate(
            [[0], np.cumsum(np.bincount(bin_of, minlength=N_BINS))]
        )[bin_of[order2]]
        assert col.max() < P
        node_slot[nodes] = bin_of * P + col
        cores.append((nodes, bin_of, col, cnt))

    prof = np.zeros(N_BINS, np.int64)
    for _, _, _, cnt in cores:
        prof = np.maximum(prof, np.ceil(cnt / P).astype(np.int64))
    profile = tuple(int(t) for t in prof)
    ttot = int(sum(profile))
    toff = np.concatenate([[0], np.cumsum(prof)])

    xd = (np.asarray(x, np.float64) * dinv[:, None]).astype(np.float32)
    # fp8-quantized destination scales (must match what the fp8 S delivers)
    dq8 = dinv.astype(np.float32).astype(ml_dtypes.float8_e4m3).astype(np.float32)

    in_maps = []
    for c in range(N_CORES):
        nodes, bin_of, col, cnt = cores[c]
        e_mask = node_core[dst] == c
        e_src = src[e_mask]
        e_dst = dst[e_mask]
        e_bin = node_slot[e_dst] // P
        e_col = node_slot[e_dst] % P
        eorder = np.argsort(e_bin, kind="stable")
        e_src, e_dst, e_bin, e_col = (
            e_src[eorder], e_dst[eorder], e_bin[eorder], e_col[eorder],
        )
        within = np.arange(len(e_src)) - np.concatenate(
            [[0], np.cumsum(np.bincount(e_bin, minlength=N_BINS))]
        )[e_bin]
        tile = toff[e_bin] + within // P
        pos = within % P

        uniq, inv = np.unique(e_src, return_inverse=True)
        assert len(uniq) < TABLE_ROWS, len(uniq)
        table = np.zeros((TABLE_ROWS, WIDTH), ml_dtypes.bfloat16)
        table[: len(uniq)] = xd[uniq]

        idx_flat = np.zeros(ttot * P, np.int16)
        idx_flat[tile * P + pos] = inv.astype(np.int16)
        a = idx_flat.reshape(ttot * 8, 16).transpose(1, 0)
        idx = np.ascontiguousarray(np.tile(a, (8, 1)))

        s_all = np.zeros((P, ttot * P), ml_dtypes.float8_e4m3)
        s_all[pos, tile * P + e_col] = dinv[e_dst].astype(np.float32)

        slot_node = np.full(DST_SLOTS, -1, np.int64)
        slot_node[bin_of * P + col] = nodes
        valid = slot_node >= 0
        xs = np.zeros((DST_SLOTS, WIDTH), np.float32)
        vn = slot_node[valid]
        # xd already carries one dinv factor; the extra dq8 factor matches
        # the fp8-quantized dst scale the edge path gets through S.
        xs[valid] = xd[vn] * dq8[vn][:, None]
        # identity tile prepended to S so the self term enters the scatter
        # PSUM as xself^T = xself_rows^T @ I
        ident = np.zeros((P, P), ml_dtypes.float8_e4m3)
        ident[np.arange(P), np.arange(P)] = 1.0
        in_maps.append(
            {
                "tab": table,
                "idx": idx,
                "sall": np.ascontiguousarray(
                    np.concatenate([ident, s_all], axis=1)
                ),
                "xself": np.ascontiguousarray(xs.astype(ml_dtypes.bfloat16)),
                "_slot_node": slot_node,
            }
        )
    return profile, in_maps


def _build_program(profile, generic_affine):
    import concourse.bass as bass
    import concourse.tile as tile
    from concourse import bacc as bacc_mod
    from concourse import mybir
    from contextlib import ExitStack

    f32 = mybir.dt.float32
    bf16 = mybir.dt.bfloat16
    fp8 = mybir.dt.float8e4
    i16 = mybir.dt.int16
    Alu = mybir.AluOpType
    Act = mybir.ActivationFunctionType
    ttot = int(sum(profile))
    toff = np.concatenate([[0], np.cumsum(profile)]).astype(np.int64)
    HEAD_BINS = 4
    thead = int(toff[HEAD_BINS])
    n_groups = math.ceil(N_BINS / SGROUP)

    nc = bacc_mod.Bacc(None, target_bir_lowering=False, debug=False, num_swdge_queues=4)
    tab_d = nc.declare_dram_parameter("tab", [TABLE_ROWS, WIDTH], bf16, isOutput=False)
    idx_d = nc.declare_dram_parameter("idx", [P, 8 * ttot], i16, isOutput=False)
    sall_d = nc.declare_dram_parameter("sall", [P, (1 + ttot) * P], fp8, isOutput=False)
    xself_d = nc.declare_dram_parameter("xself", [DST_SLOTS, WIDTH], bf16, isOutput=False)
    wt_d = nc.declare_dram_parameter("wt", [P, 2 * (WIDTH + 1)], bf16, isOutput=False)
    bcon_d = nc.declare_dram_parameter("bcon", [1, WIDTH + 1], bf16, isOutput=False)
    if generic_affine:
        fcon_d = nc.declare_dram_parameter("fcon", [P, 2 * WIDTH], f32, isOutput=False)
    out_d = nc.declare_dram_parameter("out", [DST_SLOTS, WIDTH], bf16, isOutput=True)

    with tile.TileContext(nc) as tc:
        with ExitStack() as ctx:
            const = ctx.enter_context(tc.tile_pool(name="const", bufs=1))
            gpool = ctx.enter_context(tc.tile_pool(name="g", bufs=6))
            spool = ctx.enter_context(tc.tile_pool(name="s", bufs=4))
            xpool = ctx.enter_context(tc.tile_pool(name="xs", bufs=4))
            apool = ctx.enter_context(tc.tile_pool(name="aggT", bufs=2))
            ypool = ctx.enter_context(tc.tile_pool(name="y", bufs=3))
            stat = ctx.enter_context(tc.tile_pool(name="stat", bufs=4))
            ppool = ctx.enter_context(tc.tile_pool(name="psA", bufs=2, space="PSUM"))
            opsum = ctx.enter_context(tc.tile_pool(name="psO", bufs=2, space="PSUM"))

            idx_a = const.tile([P, 8 * thead], i16)
            nc.sync.dma_start(idx_a[:], idx_d[:, : 8 * thead])
            idx_b = const.tile([P, 8 * (ttot - thead)], i16)
            nc.scalar.dma_start(idx_b[:], idx_d[:, 8 * thead :])
            wt_sb = const.tile([P, 2 * (WIDTH + 1)], bf16)
            nc.scalar.dma_start(wt_sb[:], wt_d[:, :])
            bcon_sb = const.tile([1, WIDTH + 1], bf16)
            nc.scalar.dma_start(bcon_sb[:], bcon_d[:, :])
            ident_sb = const.tile([P, P], fp8)
            nc.scalar.dma_start(ident_sb[:], sall_d[:, :P])
            ones_sb = const.tile([1, P], bf16)
            nc.vector.memset(ones_sb[:], 1.0)
            eps_sb = const.tile([P, 1], f32)
            nc.vector.memset(eps_sb[:], LN_EPS)
            if generic_affine:
                fcon_sb = const.tile([P, 2 * WIDTH], f32)
                nc.scalar.dma_start(fcon_sb[:], fcon_d[:, :])
                gamma_sb = fcon_sb[:, :WIDTH]
                beta_sb = fcon_sb[:, WIDTH : 2 * WIDTH]

            sgroups = [None] * n_groups

            for b in range(N_BINS):
                nt = int(profile[b])
                t0 = int(toff[b])
                g = gpool.tile([P, nt, WIDTH], bf16, tag="g")
                if t0 + nt <= thead:
                    isb, ioff = idx_a, t0
                else:
                    isb, ioff = idx_b, t0 - thead
                nc.gpsimd.dma_gather(
                    g[:],
                    tab_d[:, :],
                    isb[:, 8 * ioff : 8 * (ioff + nt)],
                    nt * P,
                    nt * P,
                    WIDTH,
                    queue_num=b % 4,
                )
                gb = b // SGROUP
                if sgroups[gb] is None:
                    b0 = gb * SGROUP
                    b1 = min(b0 + SGROUP, N_BINS)
                    gt0, gt1 = int(toff[b0]), int(toff[b1])
                    s = spool.tile([P, (gt1 - gt0) * P], fp8, tag="s")
                    nc.sync.dma_start(
                        s[:], sall_d[:, (1 + gt0) * P : (1 + gt1) * P]
                    )
                    sgroups[gb] = (s, gt0)
                s, gt0 = sgroups[gb]
                soff = (t0 - gt0) * P
                xself = xpool.tile([P, WIDTH], bf16, tag="xself")
                nc.sync.dma_start(xself[:], xself_d[b * P : (b + 1) * P, :])

                ps0 = ppool.tile([P, P], f32, tag="ps0")
                ps1 = ppool.tile([P, P], f32, tag="ps1")
                nc.tensor.matmul(
                    out=ps0[:], lhsT=xself[:, 0:P], rhs=ident_sb[:],
                    start=True, stop=False,
                )
                nc.tensor.matmul(
                    out=ps1[:], lhsT=xself[:, P:WIDTH], rhs=ident_sb[:],
                    start=True, stop=False,
                )
                for k in range(nt):
                    nc.tensor.matmul(
                        out=ps0[:],
                        lhsT=g[:, k, 0:P],
                        rhs=s[:, soff + k * P : soff + (k + 1) * P],
                        start=False,
                        stop=(k == nt - 1),
                    )
                    nc.tensor.matmul(
                        out=ps1[:],
                        lhsT=g[:, k, P:WIDTH],
                        rhs=s[:, soff + k * P : soff + (k + 1) * P],
                        start=False,
                        stop=(k == nt - 1),
                    )
                a0 = apool.tile([P, P], bf16, tag="a0")
                nc.vector.tensor_scalar(
                    out=a0[:], in0=ps0[:], scalar1=1.0, scalar2=None, op0=Alu.mult
                )
                a1 = apool.tile([P, P], bf16, tag="a1")
                nc.vector.tensor_scalar(
                    out=a1[:], in0=ps1[:], scalar1=1.0, scalar2=None, op0=Alu.mult
                )
                po = opsum.tile([P, WIDTH + 1], f32, tag="po")
                nc.tensor.matmul(
                    out=po[:], lhsT=ones_sb[:], rhs=bcon_sb[:],
                    start=True, stop=False,
                )
                nc.tensor.matmul(
                    out=po[:], lhsT=a0[:], rhs=wt_sb[:, : WIDTH + 1],
                    start=False, stop=False,
                )
                nc.tensor.matmul(
                    out=po[:], lhsT=a1[:], rhs=wt_sb[:, WIDTH + 1 :],
                    start=False, stop=True,
                )
                # ---- epilogue: LayerNorm(po) + ReLU (bias already in po) ----
                sq = ypool.tile([P, WIDTH], f32, tag="sq")
                ssq = stat.tile([P, 1], f32, tag="ssq")
                nc.scalar.activation(
                    out=sq[:], in_=po[:, :WIDTH], func=Act.Square, accum_out=ssq[:]
                )
                mu = stat.tile([P, 1], f32, tag="mu")
                nc.vector.tensor_scalar(
                    out=mu[:],
                    in0=po[:, WIDTH : WIDTH + 1],
                    scalar1=1.0 / WIDTH,
                    scalar2=None,
                    op0=Alu.mult,
                )
                m2 = stat.tile([P, 1], f32, tag="m2")
                nc.vector.tensor_scalar(
                    out=m2[:], in0=mu[:], scalar1=mu[:, :1], scalar2=None,
                    op0=Alu.mult,
                )
                var = stat.tile([P, 1], f32, tag="var")
                nc.vector.tensor_scalar(
                    out=var[:],
                    in0=ssq[:],
                    scalar1=1.0 / WIDTH,
                    scalar2=m2[:, :1],
                    op0=Alu.mult,
                    op1=Alu.subtract,
                )
                sd = stat.tile([P, 1], f32, tag="sd")
                nc.scalar.activation(
                    out=sd[:], in_=var[:], func=Act.Sqrt, bias=eps_sb[:, :1]
                )
                rstd = stat.tile([P, 1], f32, tag="rstd")
                nc.vector.reciprocal(rstd[:], sd[:])
                rm = stat.tile([P, 1], f32, tag="rm")
                nc.vector.tensor_scalar(
                    out=rm[:],
                    in0=mu[:],
                    scalar1=rstd[:, :1],
                    scalar2=-1.0,
                    op0=Alu.mult,
                    op1=Alu.mult,
                )
                if generic_affine:
                    t1 = ypool.tile([P, WIDTH], f32, tag="t1")
                    nc.scalar.activation(
                        out=t1[:], in_=po[:, :WIDTH], func=Act.Identity,
                        scale=rstd[:, :1], bias=rm[:, :1],
                    )
                    t2 = ypool.tile([P, WIDTH], f32, tag="t2")
                    nc.vector.tensor_tensor(out=t2[:], in0=t1[:], in1=gamma_sb, op=Alu.mult)
                    t3 = ypool.tile([P, WIDTH], f32, tag="t3")
                    nc.vector.tensor_tensor(out=t3[:], in0=t2[:], in1=beta_sb, op=Alu.add)
                    yo = ypool.tile([P, WIDTH], bf16, tag="yo")
                    nc.scalar.activation(out=yo[:], in_=t3[:], func=Act.Relu)
                else:
                    yo = ypool.tile([P, WIDTH], bf16, tag="yo")
                    nc.scalar.activation(
                        out=yo[:], in_=po[:, :WIDTH], func=Act.Relu,
                        scale=rstd[:, :1], bias=rm[:, :1],
                    )
                nc.sync.dma_start(out_d[b * P : (b + 1) * P, :], yo[:])
    return nc


_PROGRAM_CACHE = {}
_PREP_CACHE = {}


def kernel(x, edge_index, W, b, gamma, beta, _run_kwargs=None):
    from concourse.bass_utils import run_bass_kernel_spmd

    x = np.asarray(x)
    W = np.asarray(W)
    bias = np.asarray(b, dtype=np.float64)
    gamma = np.asarray(gamma)
    beta = np.asarray(beta)

    ekey = hash(np.asarray(edge_index).tobytes()) ^ hash(x.tobytes())
    if ekey not in _PREP_CACHE:
        dst = np.asarray(edge_index[1]).astype(np.int64)
        deg = np.bincount(dst, minlength=N_NODES).astype(np.float64) + 1.0
        dinv = 1.0 / np.sqrt(deg)
        _PREP_CACHE.clear()
        _PREP_CACHE[ekey] = _preprocess(edge_index, x, dinv)
    profile, in_maps = _PREP_CACHE[ekey]

    generic_affine = not (np.all(gamma == 1.0) and np.all(beta == 0.0))
    key = (profile, generic_affine)
    if key not in _PROGRAM_CACHE:
        nc = _build_program(profile, generic_affine)
        nc.finalize()
        _PROGRAM_CACHE[key] = nc
    nc = _PROGRAM_CACHE[key]

    WT32 = W.T.astype(np.float32)
    rs = WT32.sum(axis=1, keepdims=True)
    WTe = np.concatenate([WT32, rs], axis=1).astype(ml_dtypes.bfloat16)
    wt = np.ascontiguousarray(np.concatenate([WTe[:P], WTe[P:]], axis=1))
    bcon = np.concatenate([bias, [bias.sum()]]).astype(ml_dtypes.bfloat16)[None, :]
    bcon = np.ascontiguousarray(bcon)

    run_maps = []
    for c in range(N_CORES):
        m = {k: v for k, v in in_maps[c].items() if not k.startswith("_")}
        m["wt"] = wt
        m["bcon"] = bcon
        if generic_affine:
            m["fcon"] = np.ascontiguousarray(
                np.concatenate(
                    [
                        np.tile(gamma.astype(np.float32)[None, :], (P, 1)),
                        np.tile(beta.astype(np.float32)[None, :], (P, 1)),
                    ],
                    axis=1,
                )
            )
        run_maps.append(m)

    kwargs = dict(_run_kwargs or {})
    kwargs.pop("_result", None)
    rr = run_bass_kernel_spmd(nc, run_maps, list(range(N_CORES)), **kwargs)
    out = np.zeros((N_NODES, WIDTH), np.float32)
    for c in range(N_CORES):
        slot_node = in_maps[c]["_slot_node"]
        valid = slot_node >= 0
        out[slot_node[valid]] = rr.results[c]["out"][valid].astype(np.float32)
    if _run_kwargs is not None:
        _run_kwargs["_result"] = rr
    return out
